# revision 1
# baseline (speedup 1.0000x reference)
"""2-layer GAT on Trainium2 (8 NeuronCores), self-contained.

Sharding: edges partitioned by dst range (core r owns dst in [r*12500,(r+1)*12500)).
Per layer: node-phase matmul builds per-node tables [h | alpha_src] (+ [alpha_dst]
local), AllGather replicates the src-side table, edge phase dma_gathers per-edge
rows, computes attention with a one-hot(dst-offset) matmul trick that performs
the segment softmax denominator and the weighted aggregation in the same PSUM
accumulation. Layer 2 ends with head-mean + bias + log_softmax.
"""
import os
import numpy as np
import ml_dtypes

import concourse.bacc as bacc
import concourse.mybir as mybir
import concourse.tile as tile
from concourse.bass_utils import run_bass_kernel_spmd

BF16 = ml_dtypes.bfloat16

N_NODES = 100000
N_EDGES = 1600000
R = 8
NLOC = N_NODES // R            # 12500
WIN = 128
NWIN = (NLOC + WIN - 1) // WIN  # 98
NEG_SLOPE = 0.2
EPS = 1e-16
NBUCK = 4
BUCK = 25600                   # int16-safe src bucket width
CALL_MAX = 8                   # chunks per dma_gather call (1024 idxs)
SUPER = 3                      # windows per gather group
PAD_DOFF = 20000.0
TRIM = os.environ.get("K_TRIM", "0") == "1"
ZINIT = os.environ.get("K_ZINIT", "0") == "1"


def _ceil(a, b):
    return (a + b - 1) // b


def _build_layout(src, dst):
    """Static edge layout, uniform across cores: per (window, src-bucket) cell,
    dst-sorted edges in chunks of 128 lanes. Cell-tail pad slots get idx -1 so
    the gather ucode's trailing trim skips their descriptors entirely."""
    core_of = dst // NLOC
    per_core = []
    cnt = np.zeros((R, NWIN, NBUCK), dtype=np.int64)
    for r in range(R):
        sel = np.nonzero(core_of == r)[0]
        s_ = src[sel]
        d_ = dst[sel] - r * NLOC
        w_of = d_ // WIN
        b_of = s_ // BUCK
        order = np.lexsort((d_, b_of, w_of))
        s_, d_, w_of, b_of = s_[order], d_[order], w_of[order], b_of[order]
        np.add.at(cnt[r], (w_of, b_of), 1)
        per_core.append((s_, d_, w_of, b_of))

    nchwb = np.zeros((NWIN, NBUCK), dtype=np.int64)
    for w in range(NWIN):
        for b in range(NBUCK):
            nchwb[w, b] = max(_ceil(int(cnt[r, w, b]), WIN) for r in range(R))
    nch = nchwb.sum(axis=1)
    TCH = int(nch.sum())
    chunk_bucket = np.zeros(TCH, dtype=np.int64)
    gc = 0
    for w in range(NWIN):
        for b in range(NBUCK):
            chunk_bucket[gc:gc + nchwb[w, b]] = b
            gc += int(nchwb[w, b])

    # payload call boundaries (must mirror build_kernel): per super-group,
    # runs of same-bucket chunks, <= CALL_MAX
    calls = []   # (gc_start, n_chunks)
    gci = 0
    w = 0
    while w < NWIN:
        nwg = min(SUPER, NWIN - w)
        nch_g = int(nch[w:w + nwg].sum())
        c = 0
        while c < nch_g:
            b = chunk_bucket[gci + c]
            ce = c
            while ce < nch_g and chunk_bucket[gci + ce] == b and ce - c < CALL_MAX:
                ce += 1
            calls.append((gci + c, ce - c))
            c = ce
        gci += nch_g
        w += nwg

    cores = []
    for r in range(R):
        s_, d_, w_of, b_of = per_core[r]
        srcoff = np.full((128, TCH), -1 if TRIM else 0, dtype=np.int64)
        dstloc = np.zeros((128, TCH), dtype=np.int64)
        dstoff = np.full((128, TCH), PAD_DOFF, dtype=np.float32)
        occ = np.zeros(TCH, dtype=np.int64)
        wb = w_of * NBUCK + b_of
        bounds = np.searchsorted(wb, np.arange(NWIN * NBUCK + 1))
        gc = 0
        for w in range(NWIN):
            for b in range(NBUCK):
                lo, hi = bounds[w * NBUCK + b], bounds[w * NBUCK + b + 1]
                for ci in range(int(nchwb[w, b])):
                    e0 = lo + ci * WIN
                    k = max(0, min(WIN, hi - e0))
                    occ[gc] = k
                    if k > 0:
                        srcoff[:k, gc] = s_[e0:e0 + k] - b * BUCK
                        dstloc[:k, gc] = d_[e0:e0 + k]
                        dstoff[:k, gc] = d_[e0:e0 + k] - w * WIN
                    gc += 1
        # per-call valid count: trailing -1s trimmed by the ucode. Within a
        # call, pads are only in its final chunks (cell tail), so valid =
        # (sum of occ) provided every partially-filled chunk is final.
        call_valid = []
        for (g0, k) in calls:
            o = occ[g0:g0 + k]
            # prefix property check: all full before first partial
            nonfull = np.nonzero(o < WIN)[0]
            if len(nonfull) and (o[nonfull[0]:] > 0).any() and nonfull[0] < k - 1:
                # partial chunk followed by nonempty chunks can only happen at
                # cell boundary, which call splitting never crosses
                assert (o[nonfull[0] + 1:] == 0).all(), (o, g0, k)
            call_valid.append(int(o.sum()))
        def wrap(a64, ncols):
            a = a64.astype(np.int16)
            w16 = a.T.reshape(ncols, 8, 16).transpose(2, 0, 1).reshape(16, ncols * 8)
            return np.tile(w16, (8, 1))
        cores.append(dict(
            pay_idx=wrap(srcoff, TCH), adst_idx=wrap(dstloc, TCH),
            dstoff=np.ascontiguousarray(dstoff.astype(BF16)),
            call_valid=np.array(call_valid, dtype=np.int64),
        ))
    call_valid_u = np.stack([c["call_valid"] for c in cores], axis=0).max(axis=0)
    return nch, nchwb, TCH, chunk_bucket, calls, call_valid_u, cores


def build_kernel(nch, nchwb, TCH, chunk_bucket, call_valid):
    dt = mybir.dt
    f32r = dt.float32r
    nc = bacc.Bacc(None, target_bir_lowering=False, debug=True,
                   num_swdge_queues=4)

    xT = nc.dram_tensor("xT", [128, NLOC], dt.float32r, kind="ExternalInput")
    w1p = nc.dram_tensor("w1p", [128, 256], dt.float32r, kind="ExternalInput")
    w2p = nc.dram_tensor("w2p", [64, 256], dt.float32r, kind="ExternalInput")
    b1rep = nc.dram_tensor("b1rep", [128, 64], dt.float32, kind="ExternalInput")
    b2rep = nc.dram_tensor("b2rep", [128, 20], dt.float32, kind="ExternalInput")
    iota = nc.dram_tensor("iota", [128, 128], dt.bfloat16, kind="ExternalInput")
    ident = nc.dram_tensor("ident", [128, 128], dt.float32, kind="ExternalInput")
    doff_in = nc.dram_tensor("doff", [128, TCH], dt.bfloat16, kind="ExternalInput")
    pay_idx = nc.dram_tensor("pay_idx", [128, TCH * 8], dt.int16, kind="ExternalInput")
    adst_idx = nc.dram_tensor("adst_idx", [128, TCH * 8], dt.int16, kind="ExternalInput")
    out_ext = nc.dram_tensor("out", [NLOC, 20], dt.float32, kind="ExternalOutput")

    ts1_loc = nc.dram_tensor("ts1_loc", [NLOC, 128], dt.bfloat16)
    ts1_full = nc.dram_tensor("ts1_full", [N_NODES, 128], dt.bfloat16, addr_space="Shared")
    td1_loc = nc.dram_tensor("td1_loc", [NLOC, 128], dt.bfloat16)
    ts2_loc = nc.dram_tensor("ts2_loc", [NLOC, 256], dt.bfloat16)
    ts2_full = nc.dram_tensor("ts2_full", [N_NODES, 256], dt.bfloat16, addr_space="Shared")
    td2_loc = nc.dram_tensor("td2_loc", [NLOC, 128], dt.bfloat16)

    NCHMAX = int(nch.max())

    with tile.TileContext(nc) as tc:
        with (
            tc.tile_pool(name="const", bufs=1) as cpool,
            tc.tile_pool(name="sb", bufs=2) as sb,
            tc.tile_pool(name="gb", bufs=2) as gb,
            tc.tile_pool(name="ps", bufs=2, space="PSUM") as ps,
        ):
            w1t = cpool.tile([128, 256], dt.float32r, name="w1t")
            nc.sync.dma_start(out=w1t[:], in_=w1p[:])
            w2t = cpool.tile([64, 256], dt.float32r, name="w2t")
            nc.sync.dma_start(out=w2t[:], in_=w2p[:])
            b1t = cpool.tile([128, 64], dt.float32, name="b1t")
            nc.sync.dma_start(out=b1t[:], in_=b1rep[:])
            b2t = cpool.tile([128, 20], dt.float32, name="b2t")
            nc.sync.dma_start(out=b2t[:], in_=b2rep[:])
            iot = cpool.tile([128, 128], dt.bfloat16, name="iot")
            nc.sync.dma_start(out=iot[:], in_=iota[:])
            idt = cpool.tile([128, 128], dt.float32, name="idt")
            nc.sync.dma_start(out=idt[:], in_=ident[:])
            dofft = cpool.tile([128, TCH], dt.bfloat16, name="dofft")
            nc.sync.dma_start(out=dofft[:], in_=doff_in[:])

            # ---------- Phase A: T1 rows for local nodes ----------
            for w in range(NWIN):
                nw = min(WIN, NLOC - w * WIN)
                xt = sb.tile([128, WIN], dt.float32r, name=f"xt{w}", tag="xt")
                nc.sync.dma_start(out=xt[:, :nw], in_=xT[:, w * WIN:w * WIN + nw])
                pA = ps.tile([WIN, 256], dt.float32, name=f"pA{w}", tag="pA")
                nc.tensor.matmul(out=pA[:nw, :], lhsT=xt[:, :nw],
                                 rhs=w1t[:], start=True, stop=True)
                t1 = sb.tile([WIN, 80], dt.bfloat16, name=f"t1o{w}", tag="t1o")
                nc.vector.tensor_copy(out=t1[:nw, :], in_=pA[:nw, 0:80])
                nc.sync.dma_start(out=ts1_loc[w * WIN:w * WIN + nw, 0:72],
                                  in_=t1[:nw, 0:72])
                nc.sync.dma_start(out=td1_loc[w * WIN:w * WIN + nw, 0:8],
                                  in_=t1[:nw, 72:80])

            nc.gpsimd.collective_compute(
                "AllGather", mybir.AluOpType.bypass,
                replica_groups=[list(range(R))],
                ins=[ts1_loc[:]], outs=[ts1_full[:]])

            # ---------- window epilogues ----------
            def epilogue1(wi, nw, pagg):
                t8 = sb.tile([WIN, 8], dt.float32, name=f"t8a{wi}", tag="t8a")
                nc.vector.tensor_scalar(t8[:nw], pagg[:nw, 64:72], EPS, None,
                                        mybir.AluOpType.add)
                inv8 = sb.tile([WIN, 8], dt.float32, name=f"i8a{wi}", tag="i8a")
                nc.vector.reciprocal(inv8[:nw], t8[:nw])
                z = sb.tile([WIN, 64], dt.float32, name=f"z{wi}", tag="z")
                nc.vector.tensor_tensor(
                    out=z[:nw].rearrange("p (h e) -> p h e", e=8),
                    in0=pagg[:nw, 0:64].rearrange("p (h e) -> p h e", e=8),
                    in1=inv8[:nw].unsqueeze(2).broadcast_to([nw, 8, 8]),
                    op=mybir.AluOpType.mult)
                nc.vector.tensor_tensor(out=z[:nw], in0=z[:nw], in1=b1t[:nw],
                                        op=mybir.AluOpType.add)
                mz = sb.tile([WIN, 64], dt.float32, name=f"mz{wi}", tag="mz")
                nc.vector.tensor_scalar(mz[:nw], z[:nw], 0.0, None,
                                        mybir.AluOpType.min)
                nc.scalar.activation(mz[:nw], mz[:nw],
                                     mybir.ActivationFunctionType.Exp)
                rz = sb.tile([WIN, 64], dt.float32, name=f"rz{wi}", tag="rz")
                nc.scalar.activation(rz[:nw], z[:nw],
                                     mybir.ActivationFunctionType.Relu)
                nc.vector.tensor_tensor(out=z[:nw], in0=mz[:nw], in1=rz[:nw],
                                        op=mybir.AluOpType.add)
                h2 = sb.tile([WIN, 64], dt.float32, name=f"h2{wi}", tag="h2")
                nc.vector.tensor_scalar(h2[:nw], z[:nw], -1.0, None,
                                        mybir.AluOpType.add)
                ptr = ps.tile([64, WIN], dt.float32, name=f"ptr{wi}", tag="ptr")
                nc.tensor.transpose(out=ptr[:, :nw], in_=h2[:nw, :], identity=idt[:nw, :nw])
                h2T = sb.tile([64, WIN], dt.float32r, name=f"h2T{wi}", tag="h2T")
                nc.scalar.copy(out=h2T[:, :nw], in_=ptr[:, :nw])
                pT2 = ps.tile([WIN, 256], dt.float32, name=f"pT2{wi}", tag="pA")
                nc.tensor.matmul(out=pT2[:nw, :], lhsT=h2T[:, :nw],
                                 rhs=w2t[:], start=True, stop=True)
                t2o = sb.tile([WIN, 176], dt.bfloat16, name=f"t2o{wi}", tag="t2o")
                nc.vector.tensor_copy(out=t2o[:nw, :], in_=pT2[:nw, 0:176])
                nc.sync.dma_start(out=ts2_loc[wi * WIN:wi * WIN + nw, 0:168],
                                  in_=t2o[:nw, 0:168])
                nc.sync.dma_start(out=td2_loc[wi * WIN:wi * WIN + nw, 0:8],
                                  in_=t2o[:nw, 168:176])

            def epilogue2(wi, nw, pagg):
                t8 = sb.tile([WIN, 8], dt.float32, name=f"t8b{wi}", tag="t8b")
                nc.vector.tensor_scalar(t8[:nw], pagg[:nw, 160:168], EPS, 8.0,
                                        mybir.AluOpType.add, mybir.AluOpType.mult)
                inv8 = sb.tile([WIN, 8], dt.float32, name=f"i8b{wi}", tag="i8b")
                nc.vector.reciprocal(inv8[:nw], t8[:nw])
                v = sb.tile([WIN, 20], dt.float32, name=f"v{wi}", tag="v")
                val = sb.tile([WIN, 160], dt.float32, name=f"val{wi}", tag="val")
                nc.vector.tensor_tensor(
                    out=val[:nw].rearrange("p (c h) -> p c h", h=8),
                    in0=pagg[:nw, 0:160].rearrange("p (c h) -> p c h", h=8),
                    in1=inv8[:nw].unsqueeze(1).broadcast_to([nw, 20, 8]),
                    op=mybir.AluOpType.mult)
                nc.vector.tensor_reduce(
                    out=v[:nw], in_=val[:nw].rearrange("p (c h) -> p c h", h=8),
                    axis=mybir.AxisListType.X, op=mybir.AluOpType.add)
                nc.vector.tensor_tensor(out=v[:nw], in0=v[:nw], in1=b2t[:nw],
                                        op=mybir.AluOpType.add)
                mx = sb.tile([WIN, 1], dt.float32, name=f"mx{wi}", tag="mx")
                nc.vector.tensor_reduce(out=mx[:nw], in_=v[:nw],
                                        axis=mybir.AxisListType.X,
                                        op=mybir.AluOpType.max)
                nmx = sb.tile([WIN, 1], dt.float32, name=f"nmx{wi}", tag="nmx")
                nc.vector.tensor_scalar(nmx[:nw], mx[:nw], -1.0, None,
                                        mybir.AluOpType.mult)
                ex = sb.tile([WIN, 20], dt.float32, name=f"exo{wi}", tag="exo")
                ssum = sb.tile([WIN, 1], dt.float32, name=f"ss{wi}", tag="ss")
                nc.scalar.activation(ex[:nw], v[:nw],
                                     mybir.ActivationFunctionType.Exp,
                                     bias=nmx[:nw], accum_out=ssum[:nw])
                lse = sb.tile([WIN, 1], dt.float32, name=f"lse{wi}", tag="lse")
                nc.scalar.activation(lse[:nw], ssum[:nw],
                                     mybir.ActivationFunctionType.Ln)
                nc.vector.tensor_tensor(
                    out=ex[:nw], in0=v[:nw],
                    in1=nmx[:nw].broadcast_to([nw, 20]),
                    op=mybir.AluOpType.add)
                ot = sb.tile([WIN, 20], dt.float32, name=f"ot{wi}", tag="ot")
                nc.vector.tensor_tensor(
                    out=ot[:nw], in0=ex[:nw],
                    in1=lse[:nw].broadcast_to([nw, 20]),
                    op=mybir.AluOpType.subtract)
                nc.sync.dma_start(out=out_ext[wi * WIN:wi * WIN + nw, :],
                                  in_=ot[:nw, :])

            # ---------- slot init: zero the two gbuf pool slots so lanes
            # skipped by the gather trim read finite stale data ----------
            if ZINIT:
                for ii in range(2):
                    gz = gb.tile([128, NCHMAX * SUPER, 256], dt.bfloat16,
                                 name=f"gz{ii}", tag="gbuf")
                    nc.scalar.memzero(gz[:])

            # ---------- edge phase ----------
            call_counter = [0]

            def edge_phase(layer):
                if layer == 1:
                    tsrc_full, tdst_loc = ts1_full, td1_loc
                    EW, CY, CH, NH_IN = 128, 72, 64, 8
                else:
                    tsrc_full, tdst_loc = ts2_full, td2_loc
                    EW, CY, CH, NH_IN = 256, 168, 160, 20

                gc0 = 0
                qn = 0
                w = 0
                while w < NWIN:
                    nwg = min(SUPER, NWIN - w)
                    nch_g = int(nch[w:w + nwg].sum())
                    gbuf = gb.tile([128, nch_g, EW], dt.bfloat16,
                                   name=f"gbuf{layer}_{w}", tag="gbuf",
                                   padded_shape=[128, NCHMAX * SUPER * (256 // EW), EW])
                    abuf = gb.tile([128, nch_g, 128], dt.bfloat16,
                                   name=f"abuf{layer}_{w}", tag="abuf",
                                   padded_shape=[128, NCHMAX * SUPER, 128])
                    c = 0
                    while c < nch_g:
                        b = int(chunk_bucket[gc0 + c])
                        ce = c
                        while (ce < nch_g and int(chunk_bucket[gc0 + ce]) == b
                               and ce - c < CALL_MAX):
                            ce += 1
                        k = ce - c
                        it = sb.tile([128, k * 8], dt.int16,
                                     name=f"pi{layer}_{w}_{c}", tag="pidx",
                                     padded_shape=[128, CALL_MAX * 8])
                        nc.sync.dma_start(
                            out=it[:],
                            in_=pay_idx[:, (gc0 + c) * 8:(gc0 + ce) * 8])
                        nc.gpsimd.dma_gather(
                            gbuf[:, c:ce, :],
                            tsrc_full[b * BUCK:min((b + 1) * BUCK, N_NODES), :],
                            it[:], k * 128, k * 128, EW,
                            queue_num=qn % 4, single_packet=True)
                        qn += 1
                        c = ce
                    c = 0
                    while c < nch_g:
                        k = min(CALL_MAX, nch_g - c)
                        it2 = sb.tile([128, k * 8], dt.int16,
                                      name=f"ai{layer}_{w}_{c}", tag="aidx",
                                      padded_shape=[128, CALL_MAX * 8])
                        nc.sync.dma_start(
                            out=it2[:],
                            in_=adst_idx[:, (gc0 + c) * 8:(gc0 + c + k) * 8])
                        nc.gpsimd.dma_gather(
                            abuf[:, c:c + k, :], tdst_loc[:],
                            it2[:], k * 128, k * 128, 128,
                            queue_num=qn % 4, single_packet=True)
                        qn += 1
                        c += k
                    cw = 0
                    for wi in range(w, w + nwg):
                        nchw = int(nch[wi])
                        nw_nodes = min(WIN, NLOC - wi * WIN)
                        sl = slice(cw, cw + nchw)
                        ebuf = sb.tile([128, nchw, 8], dt.float32,
                                       name=f"eb{layer}_{wi}", tag="ebuf",
                                       padded_shape=[128, NCHMAX, 8])
                        nc.vector.tensor_tensor(
                            out=ebuf[:], in0=gbuf[:, sl, CH:CH + 8],
                            in1=abuf[:, sl, 0:8], op=mybir.AluOpType.add)
                        nc.scalar.activation(ebuf[:], ebuf[:],
                                             mybir.ActivationFunctionType.Prelu,
                                             alpha=NEG_SLOPE)
                        nc.scalar.activation(gbuf[:, sl, CH:CH + 8], ebuf[:],
                                             mybir.ActivationFunctionType.Exp)
                        if layer == 1:
                            nc.vector.tensor_tensor(
                                out=gbuf[:, sl, 0:CH].rearrange(
                                    "p c (h e) -> p c h e", e=8),
                                in0=gbuf[:, sl, 0:CH].rearrange(
                                    "p c (h e) -> p c h e", e=8),
                                in1=gbuf[:, sl, CH:CH + 8].unsqueeze(3)
                                    .broadcast_to([128, nchw, 8, 8]),
                                op=mybir.AluOpType.mult)
                        else:
                            nc.vector.tensor_tensor(
                                out=gbuf[:, sl, 0:CH].rearrange(
                                    "p c (e h) -> p c e h", h=8),
                                in0=gbuf[:, sl, 0:CH].rearrange(
                                    "p c (e h) -> p c e h", h=8),
                                in1=gbuf[:, sl, CH:CH + 8].unsqueeze(2)
                                    .broadcast_to([128, nchw, 20, 8]),
                                op=mybir.AluOpType.mult)
                        oh = sb.tile([128, nchw, 128], dt.bfloat16,
                                     name=f"oh{layer}_{wi}", tag="oh",
                                     padded_shape=[128, NCHMAX, 128])
                        nc.vector.tensor_tensor(
                            out=oh[:],
                            in0=dofft[:, gc0 + cw:gc0 + cw + nchw]
                                .unsqueeze(2).broadcast_to([128, nchw, 128]),
                            in1=iot[:].unsqueeze(1).broadcast_to([128, nchw, 128]),
                            op=mybir.AluOpType.is_equal)
                        pagg = ps.tile([WIN, CY], dt.float32,
                                       name=f"pg{layer}_{wi}", tag="pagg",
                                       padded_shape=[WIN, 168])
                        for ci in range(nchw):
                            nc.tensor.matmul(
                                out=pagg[:, :],
                                lhsT=oh[:, ci, :],
                                rhs=gbuf[:, cw + ci, 0:CY],
                                start=(ci == 0), stop=(ci == nchw - 1))
                        if layer == 1:
                            epilogue1(wi, nw_nodes, pagg)
                        else:
                            epilogue2(wi, nw_nodes, pagg)
                        cw += nchw
                    gc0 += nch_g
                    w += nwg

            edge_phase(1)
            nc.gpsimd.collective_compute(
                "AllGather", mybir.AluOpType.bypass,
                replica_groups=[list(range(R))],
                ins=[ts2_loc[:]], outs=[ts2_full[:]])
            edge_phase(2)

    nc.compile()
    return nc


def _wext(w, a_src, a_dst, out_rows, pad_cols=256, permute_ch=False, heads=8):
    """[W | W.a_src | W.a_dst] padded to [out_rows, pad_cols] f32."""
    f = w.shape[0]
    c = w.shape[1] // heads
    w3 = w.reshape(f, heads, c)
    if permute_ch:
        wmain = w3.transpose(0, 2, 1).reshape(f, heads * c)  # col = ch*H + head
    else:
        wmain = w
    was = np.einsum("fhc,hc->fh", w3, a_src)
    wad = np.einsum("fhc,hc->fh", w3, a_dst)
    out = np.zeros((out_rows, pad_cols), dtype=np.float32)
    out[:f, :w.shape[1]] = wmain
    out[:f, w.shape[1]:w.shape[1] + heads] = was
    out[:f, w.shape[1] + heads:w.shape[1] + 2 * heads] = wad
    return out


_CACHE = {}


def kernel(x, edge_index, w1, att_src1, att_dst1, b1, w2, att_src2, att_dst2, b2):
    x = np.asarray(x, dtype=np.float32)
    edge_index = np.asarray(edge_index)
    src = np.concatenate([edge_index[0], np.arange(N_NODES, dtype=np.int64)]).astype(np.int64)
    dst = np.concatenate([edge_index[1], np.arange(N_NODES, dtype=np.int64)]).astype(np.int64)

    key = hash(edge_index.tobytes())
    if key not in _CACHE:
        nch, nchwb, TCH, chunk_bucket, calls, call_valid, cores = _build_layout(src, dst)
        nkern = build_kernel(nch, nchwb, TCH, chunk_bucket, call_valid)
        _CACHE[key] = (nch, TCH, cores, nkern)
    nch, TCH, cores, nkern = _CACHE[key]

    w1p = _wext(np.asarray(w1, np.float32), np.asarray(att_src1, np.float32),
                np.asarray(att_dst1, np.float32), 128)
    w2p = _wext(np.asarray(w2, np.float32), np.asarray(att_src2, np.float32),
                np.asarray(att_dst2, np.float32), 64, permute_ch=True)
    b1rep = np.tile(np.asarray(b1, np.float32)[None, :], (128, 1))
    b2rep = np.tile(np.asarray(b2, np.float32)[None, :], (128, 1))
    iota = np.broadcast_to(np.arange(128, dtype=np.float32), (128, 128)).astype(BF16)
    ident = np.eye(128, dtype=np.float32)

    in_maps = []
    for r in range(R):
        xTr = np.ascontiguousarray(x[r * NLOC:(r + 1) * NLOC].T)
        in_maps.append({
            "xT": xTr, "w1p": w1p, "w2p": w2p, "b1rep": b1rep, "b2rep": b2rep,
            "iota": np.ascontiguousarray(iota), "ident": ident,
            "doff": cores[r]["dstoff"],
            "pay_idx": cores[r]["pay_idx"], "adst_idx": cores[r]["adst_idx"],
        })

    res = run_bass_kernel_spmd(nkern, in_maps, core_ids=list(range(R)))
    out = np.concatenate([res.results[r]["out"] for r in range(R)], axis=0)
    return out.astype(np.float32)



# revision 25
# speedup vs baseline: 1.4020x; 1.4020x over previous
"""2-layer GAT on Trainium2 (8 NeuronCores), self-contained.

Sharding: edges partitioned by dst range (core r owns dst in [r*12500,(r+1)*12500)).
Per layer: node-phase matmul builds per-node tables [h | alpha_src] (+ [alpha_dst]
local), AllGather replicates the src-side table, edge phase dma_gathers per-edge
rows, computes attention with a one-hot(dst-offset) matmul trick that performs
the segment softmax denominator and the weighted aggregation in the same PSUM
accumulation. Layer 2 ends with head-mean + bias + log_softmax.
"""
import os
import numpy as np
import ml_dtypes

import concourse.bacc as bacc
import concourse.mybir as mybir
import concourse.tile as tile
from concourse.bass_utils import run_bass_kernel_spmd

BF16 = ml_dtypes.bfloat16

N_NODES = 100000
N_EDGES = 1600000
R = 8
NLOC = N_NODES // R            # 12500
WIN = 128
NWIN = (NLOC + WIN - 1) // WIN  # 98
NEG_SLOPE = 0.2
EPS = 1e-16
NBUCK = 4
BUCK = 25600                   # int16-safe src bucket width
CALL_MAX = 8                   # chunks per dma_gather call (1024 idxs)
SUPER = 3                      # windows per gather group
PAD_DOFF = 20000.0
TRIM = os.environ.get("K_TRIM", "0") == "1"
ZINIT = os.environ.get("K_ZINIT", "0") == "1"
ABL_NO_ADST = os.environ.get("K_NO_ADST", "0") == "1"
ABL_NO_PAY = os.environ.get("K_NO_PAY", "0") == "1"
ABL_NO_COLL = os.environ.get("K_NO_COLL", "0") == "1"
ABL_NO_AGG = os.environ.get("K_NO_AGG", "0") == "1"
ABL_NO_EVEC = os.environ.get("K_NO_EVEC", "0") == "1"


def _ceil(a, b):
    return (a + b - 1) // b


def _build_layout(src, dst):
    """Static edge layout, uniform across cores: per (window, src-bucket) cell,
    dst-sorted edges in chunks of 128 lanes. Cell-tail pad slots get idx -1 so
    the gather ucode's trailing trim skips their descriptors entirely."""
    core_of = dst // NLOC
    per_core = []
    cnt = np.zeros((R, NWIN, NBUCK), dtype=np.int64)
    for r in range(R):
        sel = np.nonzero(core_of == r)[0]
        s_ = src[sel]
        d_ = dst[sel] - r * NLOC
        w_of = d_ // WIN
        b_of = s_ // BUCK
        order = np.lexsort((d_, b_of, w_of))
        s_, d_, w_of, b_of = s_[order], d_[order], w_of[order], b_of[order]
        np.add.at(cnt[r], (w_of, b_of), 1)
        per_core.append((s_, d_, w_of, b_of))

    nchwb = np.zeros((NWIN, NBUCK), dtype=np.int64)
    for w in range(NWIN):
        for b in range(NBUCK):
            nchwb[w, b] = max(_ceil(int(cnt[r, w, b]), WIN) for r in range(R))
    nch = nchwb.sum(axis=1)
    TCH = int(nch.sum())
    chunk_bucket = np.zeros(TCH, dtype=np.int64)
    gc = 0
    for w in range(NWIN):
        for b in range(NBUCK):
            chunk_bucket[gc:gc + nchwb[w, b]] = b
            gc += int(nchwb[w, b])

    # payload call boundaries (must mirror build_kernel): per super-group,
    # runs of same-bucket chunks, <= CALL_MAX
    calls = []   # (gc_start, n_chunks)
    gci = 0
    w = 0
    while w < NWIN:
        nwg = min(SUPER, NWIN - w)
        nch_g = int(nch[w:w + nwg].sum())
        c = 0
        while c < nch_g:
            b = chunk_bucket[gci + c]
            ce = c
            while ce < nch_g and chunk_bucket[gci + ce] == b and ce - c < CALL_MAX:
                ce += 1
            calls.append((gci + c, ce - c))
            c = ce
        gci += nch_g
        w += nwg

    cores = []
    for r in range(R):
        s_, d_, w_of, b_of = per_core[r]
        srcoff = np.full((128, TCH), -1 if TRIM else 0, dtype=np.int64)
        dstloc = np.zeros((128, TCH), dtype=np.int64)
        dstoff = np.full((128, TCH), PAD_DOFF, dtype=np.float32)
        occ = np.zeros(TCH, dtype=np.int64)
        wb = w_of * NBUCK + b_of
        bounds = np.searchsorted(wb, np.arange(NWIN * NBUCK + 1))
        gc = 0
        for w in range(NWIN):
            for b in range(NBUCK):
                lo, hi = bounds[w * NBUCK + b], bounds[w * NBUCK + b + 1]
                for ci in range(int(nchwb[w, b])):
                    e0 = lo + ci * WIN
                    k = max(0, min(WIN, hi - e0))
                    occ[gc] = k
                    if k > 0:
                        srcoff[:k, gc] = s_[e0:e0 + k] - b * BUCK
                        dstloc[:k, gc] = d_[e0:e0 + k]
                        dstoff[:k, gc] = d_[e0:e0 + k] - w * WIN
                    gc += 1
        # per-call valid count: trailing -1s trimmed by the ucode. Within a
        # call, pads are only in its final chunks (cell tail), so valid =
        # (sum of occ) provided every partially-filled chunk is final.
        call_valid = []
        for (g0, k) in calls:
            o = occ[g0:g0 + k]
            # prefix property check: all full before first partial
            nonfull = np.nonzero(o < WIN)[0]
            if len(nonfull) and (o[nonfull[0]:] > 0).any() and nonfull[0] < k - 1:
                # partial chunk followed by nonempty chunks can only happen at
                # cell boundary, which call splitting never crosses
                assert (o[nonfull[0] + 1:] == 0).all(), (o, g0, k)
            call_valid.append(int(o.sum()))
        def wrap(a64, ncols):
            a = a64.astype(np.int16)
            w16 = a.T.reshape(ncols, 8, 16).transpose(2, 0, 1).reshape(16, ncols * 8)
            return np.tile(w16, (8, 1))
        dstoff_bf = dstoff.astype(BF16)
        cores.append(dict(
            pay_idx=wrap(srcoff, TCH),
            dstoff=np.ascontiguousarray(dstoff_bf),
            dstoffT=np.ascontiguousarray(dstoff_bf.T.reshape(1, TCH * 128)),
            call_valid=np.array(call_valid, dtype=np.int64),
        ))
    call_valid_u = np.stack([c["call_valid"] for c in cores], axis=0).max(axis=0)
    return nch, nchwb, TCH, chunk_bucket, calls, call_valid_u, cores


def build_kernel(nch, nchwb, TCH, chunk_bucket, call_valid):
    dt = mybir.dt
    f32r = dt.float32r
    nc = bacc.Bacc(None, target_bir_lowering=False, debug=True,
                   num_swdge_queues=4)

    xT = nc.dram_tensor("xT", [128, NLOC], dt.float32r, kind="ExternalInput")
    w1p = nc.dram_tensor("w1p", [128, 256], dt.float32r, kind="ExternalInput")
    w2p = nc.dram_tensor("w2p", [64, 256], dt.float32r, kind="ExternalInput")
    b1rep = nc.dram_tensor("b1rep", [128, 64], dt.float32, kind="ExternalInput")
    b2rep = nc.dram_tensor("b2rep", [128, 20], dt.float32, kind="ExternalInput")
    iota = nc.dram_tensor("iota", [128, 128], dt.bfloat16, kind="ExternalInput")
    ident = nc.dram_tensor("ident", [128, 128], dt.float32, kind="ExternalInput")
    doff_in = nc.dram_tensor("doff", [128, TCH], dt.bfloat16, kind="ExternalInput")
    doffT_in = nc.dram_tensor("doffT", [1, TCH * 128], dt.bfloat16, kind="ExternalInput")
    iotP_in = nc.dram_tensor("iotP", [128, 1], dt.bfloat16, kind="ExternalInput")
    pay_idx = nc.dram_tensor("pay_idx", [128, TCH * 8], dt.int16, kind="ExternalInput")
    out_ext = nc.dram_tensor("out", [NLOC, 20], dt.float32, kind="ExternalOutput")

    ts1_loc = nc.dram_tensor("ts1_loc", [NLOC, 128], dt.bfloat16)
    ts1_full = nc.dram_tensor("ts1_full", [N_NODES, 128], dt.bfloat16, addr_space="Shared")
    ts2_loc = nc.dram_tensor("ts2_loc", [NLOC, 256], dt.bfloat16)
    ts2_full = nc.dram_tensor("ts2_full", [N_NODES, 256], dt.bfloat16, addr_space="Shared")

    NCHMAX = int(nch.max())

    with tile.TileContext(nc) as tc:
        with (
            tc.tile_pool(name="const", bufs=1) as cpool,
            tc.tile_pool(name="sb", bufs=2) as sb,
            tc.tile_pool(name="gb", bufs=2) as gb,
            tc.tile_pool(name="ps", bufs=2, space="PSUM") as ps,
        ):
            w1t = cpool.tile([128, 256], dt.float32r, name="w1t")
            nc.sync.dma_start(out=w1t[:], in_=w1p[:])
            w2t = cpool.tile([64, 256], dt.float32r, name="w2t")
            nc.sync.dma_start(out=w2t[:], in_=w2p[:])
            b1t = cpool.tile([128, 64], dt.float32, name="b1t")
            nc.sync.dma_start(out=b1t[:], in_=b1rep[:])
            b2t = cpool.tile([128, 20], dt.float32, name="b2t")
            nc.sync.dma_start(out=b2t[:], in_=b2rep[:])
            iot = cpool.tile([128, 128], dt.bfloat16, name="iot")
            nc.sync.dma_start(out=iot[:], in_=iota[:])
            idt = cpool.tile([128, 128], dt.float32, name="idt")
            nc.sync.dma_start(out=idt[:], in_=ident[:])
            dofft = cpool.tile([128, TCH], dt.bfloat16, name="dofft")
            nc.sync.dma_start(out=dofft[:], in_=doff_in[:])
            iotP = cpool.tile([128, 1], dt.bfloat16, name="iotP")
            nc.sync.dma_start(out=iotP[:], in_=iotP_in[:])
            # per-window [node, head] attention-dst tables, SBUF-resident
            tdT1 = cpool.tile([128, NWIN * 8], dt.bfloat16, name="tdT1")
            tdT2 = cpool.tile([128, NWIN * 8], dt.bfloat16, name="tdT2")
            nc.scalar.memzero(tdT1[:])
            nc.scalar.memzero(tdT2[:])

            # ---------- Phase A: T1 rows for local nodes ----------
            for w in range(NWIN):
                nw = min(WIN, NLOC - w * WIN)
                xt = sb.tile([128, WIN], dt.float32r, name=f"xt{w}", tag="xt")
                nc.sync.dma_start(out=xt[:, :nw], in_=xT[:, w * WIN:w * WIN + nw])
                pA = ps.tile([WIN, 256], dt.float32, name=f"pA{w}", tag="pA")
                nc.tensor.matmul(out=pA[:nw, :], lhsT=xt[:, :nw],
                                 rhs=w1t[:], start=True, stop=True)
                t1 = sb.tile([WIN, 80], dt.bfloat16, name=f"t1o{w}", tag="t1o")
                nc.vector.tensor_copy(out=t1[:nw, :], in_=pA[:nw, 0:80])
                nc.sync.dma_start(out=ts1_loc[w * WIN:w * WIN + nw, 0:72],
                                  in_=t1[:nw, 0:72])
                nc.vector.tensor_copy(out=tdT1[:nw, w * 8:w * 8 + 8],
                                      in_=t1[:nw, 72:80])

            if not ABL_NO_COLL:
                nc.gpsimd.collective_compute(
                    "AllGather", mybir.AluOpType.bypass,
                    replica_groups=[list(range(R))],
                    ins=[ts1_loc[:]], outs=[ts1_full[:]])

            # ---------- window epilogues ----------
            def epilogue1(wi, nw, pagg):
                t8 = sb.tile([WIN, 8], dt.float32, name=f"t8a{wi}", tag="t8a")
                nc.vector.tensor_scalar(t8[:nw], pagg[:nw, 64:72], EPS, None,
                                        mybir.AluOpType.add)
                inv8 = sb.tile([WIN, 8], dt.float32, name=f"i8a{wi}", tag="i8a")
                nc.vector.reciprocal(inv8[:nw], t8[:nw])
                z = sb.tile([WIN, 64], dt.float32, name=f"z{wi}", tag="z")
                nc.vector.tensor_tensor(
                    out=z[:nw].rearrange("p (h e) -> p h e", e=8),
                    in0=pagg[:nw, 0:64].rearrange("p (h e) -> p h e", e=8),
                    in1=inv8[:nw].unsqueeze(2).broadcast_to([nw, 8, 8]),
                    op=mybir.AluOpType.mult)
                nc.vector.tensor_tensor(out=z[:nw], in0=z[:nw], in1=b1t[:nw],
                                        op=mybir.AluOpType.add)
                mz = sb.tile([WIN, 64], dt.float32, name=f"mz{wi}", tag="mz")
                nc.vector.tensor_scalar(mz[:nw], z[:nw], 0.0, None,
                                        mybir.AluOpType.min)
                nc.scalar.activation(mz[:nw], mz[:nw],
                                     mybir.ActivationFunctionType.Exp)
                rz = sb.tile([WIN, 64], dt.float32, name=f"rz{wi}", tag="rz")
                nc.scalar.activation(rz[:nw], z[:nw],
                                     mybir.ActivationFunctionType.Relu)
                nc.vector.tensor_tensor(out=z[:nw], in0=mz[:nw], in1=rz[:nw],
                                        op=mybir.AluOpType.add)
                h2 = sb.tile([WIN, 64], dt.float32, name=f"h2{wi}", tag="h2")
                nc.vector.tensor_scalar(h2[:nw], z[:nw], -1.0, None,
                                        mybir.AluOpType.add)
                ptr = ps.tile([64, WIN], dt.float32, name=f"ptr{wi}", tag="ptr")
                nc.tensor.transpose(out=ptr[:, :nw], in_=h2[:nw, :], identity=idt[:nw, :nw])
                h2T = sb.tile([64, WIN], dt.float32r, name=f"h2T{wi}", tag="h2T")
                nc.scalar.copy(out=h2T[:, :nw], in_=ptr[:, :nw])
                pT2 = ps.tile([WIN, 256], dt.float32, name=f"pT2{wi}", tag="pA")
                nc.tensor.matmul(out=pT2[:nw, :], lhsT=h2T[:, :nw],
                                 rhs=w2t[:], start=True, stop=True)
                t2o = sb.tile([WIN, 176], dt.bfloat16, name=f"t2o{wi}", tag="t2o")
                nc.vector.tensor_copy(out=t2o[:nw, :], in_=pT2[:nw, 0:176])
                nc.sync.dma_start(out=ts2_loc[wi * WIN:wi * WIN + nw, 0:168],
                                  in_=t2o[:nw, 0:168])
                nc.vector.tensor_copy(out=tdT2[:nw, wi * 8:wi * 8 + 8],
                                      in_=t2o[:nw, 168:176])

            def epilogue2(wi, nw, pagg):
                t8 = sb.tile([WIN, 8], dt.float32, name=f"t8b{wi}", tag="t8b")
                nc.vector.tensor_scalar(t8[:nw], pagg[:nw, 160:168], EPS, 8.0,
                                        mybir.AluOpType.add, mybir.AluOpType.mult)
                inv8 = sb.tile([WIN, 8], dt.float32, name=f"i8b{wi}", tag="i8b")
                nc.vector.reciprocal(inv8[:nw], t8[:nw])
                v = sb.tile([WIN, 20], dt.float32, name=f"v{wi}", tag="v")
                val = sb.tile([WIN, 160], dt.float32, name=f"val{wi}", tag="val")
                nc.vector.tensor_tensor(
                    out=val[:nw].rearrange("p (c h) -> p c h", h=8),
                    in0=pagg[:nw, 0:160].rearrange("p (c h) -> p c h", h=8),
                    in1=inv8[:nw].unsqueeze(1).broadcast_to([nw, 20, 8]),
                    op=mybir.AluOpType.mult)
                nc.vector.tensor_reduce(
                    out=v[:nw], in_=val[:nw].rearrange("p (c h) -> p c h", h=8),
                    axis=mybir.AxisListType.X, op=mybir.AluOpType.add)
                nc.vector.tensor_tensor(out=v[:nw], in0=v[:nw], in1=b2t[:nw],
                                        op=mybir.AluOpType.add)
                mx = sb.tile([WIN, 1], dt.float32, name=f"mx{wi}", tag="mx")
                nc.vector.tensor_reduce(out=mx[:nw], in_=v[:nw],
                                        axis=mybir.AxisListType.X,
                                        op=mybir.AluOpType.max)
                nmx = sb.tile([WIN, 1], dt.float32, name=f"nmx{wi}", tag="nmx")
                nc.vector.tensor_scalar(nmx[:nw], mx[:nw], -1.0, None,
                                        mybir.AluOpType.mult)
                ex = sb.tile([WIN, 20], dt.float32, name=f"exo{wi}", tag="exo")
                ssum = sb.tile([WIN, 1], dt.float32, name=f"ss{wi}", tag="ss")
                nc.scalar.activation(ex[:nw], v[:nw],
                                     mybir.ActivationFunctionType.Exp,
                                     bias=nmx[:nw], accum_out=ssum[:nw])
                lse = sb.tile([WIN, 1], dt.float32, name=f"lse{wi}", tag="lse")
                nc.scalar.activation(lse[:nw], ssum[:nw],
                                     mybir.ActivationFunctionType.Ln)
                nc.vector.tensor_tensor(
                    out=ex[:nw], in0=v[:nw],
                    in1=nmx[:nw].broadcast_to([nw, 20]),
                    op=mybir.AluOpType.add)
                ot = sb.tile([WIN, 20], dt.float32, name=f"ot{wi}", tag="ot")
                nc.vector.tensor_tensor(
                    out=ot[:nw], in0=ex[:nw],
                    in1=lse[:nw].broadcast_to([nw, 20]),
                    op=mybir.AluOpType.subtract)
                nc.sync.dma_start(out=out_ext[wi * WIN:wi * WIN + nw, :],
                                  in_=ot[:nw, :])

            # ---------- slot init: zero the two gbuf pool slots so lanes
            # skipped by the gather trim read finite stale data ----------
            if ZINIT:
                for ii in range(2):
                    gz = gb.tile([128, NCHMAX * SUPER, 256], dt.bfloat16,
                                 name=f"gz{ii}", tag="gbuf")
                    nc.scalar.memzero(gz[:])

            # ---------- edge phase ----------
            call_counter = [0]

            def edge_phase(layer):
                if layer == 1:
                    tsrc_full, tdT = ts1_full, tdT1
                    EW, CY, CH, NH_IN = 128, 72, 64, 8
                else:
                    tsrc_full, tdT = ts2_full, tdT2
                    EW, CY, CH, NH_IN = 256, 168, 160, 20

                gc0 = 0
                qn = 0
                w = 0
                while w < NWIN:
                    nwg = min(SUPER, NWIN - w)
                    nch_g = int(nch[w:w + nwg].sum())
                    gbuf = gb.tile([128, nch_g, EW], dt.bfloat16,
                                   name=f"gbuf{layer}_{w}", tag="gbuf",
                                   padded_shape=[128, NCHMAX * SUPER * (256 // EW), EW])

                    c = 0
                    while c < nch_g and not ABL_NO_PAY:
                        b = int(chunk_bucket[gc0 + c])
                        ce = c
                        while (ce < nch_g and int(chunk_bucket[gc0 + ce]) == b
                               and ce - c < CALL_MAX):
                            ce += 1
                        k = ce - c
                        it = sb.tile([128, k * 8], dt.int16,
                                     name=f"pi{layer}_{w}_{c}", tag="pidx",
                                     padded_shape=[128, CALL_MAX * 8])
                        nc.sync.dma_start(
                            out=it[:],
                            in_=pay_idx[:, (gc0 + c) * 8:(gc0 + ce) * 8])
                        nc.gpsimd.dma_gather(
                            gbuf[:, c:ce, :],
                            tsrc_full[b * BUCK:min((b + 1) * BUCK, N_NODES), :],
                            it[:], k * 128, k * 128, EW,
                            queue_num=qn % 4, single_packet=True)
                        qn += 1
                        c = ce
                    cw = 0
                    for wi in range(w, w + nwg):
                        nchw = int(nch[wi])
                        nw_nodes = min(WIN, NLOC - wi * WIN)
                        sl = slice(cw, cw + nchw)
                        # transposed one-hot [node, (chunk, lane)] for the
                        # per-edge adst matmul
                        dfT = sb.tile([128, nchw * 128], dt.bfloat16,
                                      name=f"dfT{layer}_{wi}", tag="dfT",
                                      padded_shape=[128, NCHMAX * 128])
                        nc.sync.dma_start(
                            out=dfT[:],
                            in_=doffT_in[0:1, (gc0 + cw) * 128:(gc0 + cw + nchw) * 128]
                                .broadcast_to([128, nchw * 128]))
                        ohT = sb.tile([128, nchw * 128], dt.bfloat16,
                                      name=f"ohT{layer}_{wi}", tag="ohT",
                                      padded_shape=[128, NCHMAX * 128])
                        nc.vector.tensor_tensor(
                            out=ohT[:],
                            in0=iotP[:, 0:1].broadcast_to([128, nchw * 128]),
                            in1=dfT[:], op=mybir.AluOpType.is_equal)
                        eadst = ps.tile([128, nchw * 8], dt.float32,
                                        name=f"ea{layer}_{wi}", tag="eadst",
                                        padded_shape=[128, NCHMAX * 8])
                        for ci in range(nchw):
                            nc.tensor.matmul(
                                out=eadst[:, ci * 8:ci * 8 + 8],
                                lhsT=ohT[:, ci * 128:ci * 128 + 128],
                                rhs=tdT[:, wi * 8:wi * 8 + 8],
                                start=True, stop=True)
                        ebuf = sb.tile([128, nchw, 8], dt.float32,
                                       name=f"eb{layer}_{wi}", tag="ebuf",
                                       padded_shape=[128, NCHMAX, 8])
                        nc.vector.tensor_tensor(
                            out=ebuf[:], in0=gbuf[:, sl, CH:CH + 8],
                            in1=eadst[:].rearrange("p (c h) -> p c h", h=8),
                            op=mybir.AluOpType.add)
                        nc.scalar.activation(ebuf[:], ebuf[:],
                                             mybir.ActivationFunctionType.Prelu,
                                             alpha=NEG_SLOPE)
                        nc.scalar.activation(gbuf[:, sl, CH:CH + 8], ebuf[:],
                                             mybir.ActivationFunctionType.Exp)
                        if layer == 1:
                            nc.vector.tensor_tensor(
                                out=gbuf[:, sl, 0:CH].rearrange(
                                    "p c (h e) -> p c h e", e=8),
                                in0=gbuf[:, sl, 0:CH].rearrange(
                                    "p c (h e) -> p c h e", e=8),
                                in1=gbuf[:, sl, CH:CH + 8].unsqueeze(3)
                                    .broadcast_to([128, nchw, 8, 8]),
                                op=mybir.AluOpType.mult)
                        else:
                            nc.vector.tensor_tensor(
                                out=gbuf[:, sl, 0:CH].rearrange(
                                    "p c (e h) -> p c e h", h=8),
                                in0=gbuf[:, sl, 0:CH].rearrange(
                                    "p c (e h) -> p c e h", h=8),
                                in1=gbuf[:, sl, CH:CH + 8].unsqueeze(2)
                                    .broadcast_to([128, nchw, 20, 8]),
                                op=mybir.AluOpType.mult)
                        oh = sb.tile([128, nchw, 128], dt.bfloat16,
                                     name=f"oh{layer}_{wi}", tag="oh",
                                     padded_shape=[128, NCHMAX, 128])
                        nc.vector.tensor_tensor(
                            out=oh[:],
                            in0=dofft[:, gc0 + cw:gc0 + cw + nchw]
                                .unsqueeze(2).broadcast_to([128, nchw, 128]),
                            in1=iot[:].unsqueeze(1).broadcast_to([128, nchw, 128]),
                            op=mybir.AluOpType.is_equal)
                        pagg = ps.tile([WIN, CY], dt.float32,
                                       name=f"pg{layer}_{wi}", tag="pagg",
                                       padded_shape=[WIN, 168])
                        nmm = 1 if ABL_NO_AGG else nchw
                        for ci in range(nmm):
                            nc.tensor.matmul(
                                out=pagg[:, :],
                                lhsT=oh[:, ci, :],
                                rhs=gbuf[:, cw + ci, 0:CY],
                                start=(ci == 0), stop=(ci == nmm - 1))
                        if layer == 1:
                            epilogue1(wi, nw_nodes, pagg)
                        else:
                            epilogue2(wi, nw_nodes, pagg)
                        cw += nchw
                    gc0 += nch_g
                    w += nwg

            edge_phase(1)
            if not ABL_NO_COLL:
                nc.gpsimd.collective_compute(
                    "AllGather", mybir.AluOpType.bypass,
                    replica_groups=[list(range(R))],
                    ins=[ts2_loc[:]], outs=[ts2_full[:]])
            edge_phase(2)

    nc.compile()
    return nc


def _wext(w, a_src, a_dst, out_rows, pad_cols=256, permute_ch=False, heads=8):
    """[W | W.a_src | W.a_dst] padded to [out_rows, pad_cols] f32."""
    f = w.shape[0]
    c = w.shape[1] // heads
    w3 = w.reshape(f, heads, c)
    if permute_ch:
        wmain = w3.transpose(0, 2, 1).reshape(f, heads * c)  # col = ch*H + head
    else:
        wmain = w
    was = np.einsum("fhc,hc->fh", w3, a_src)
    wad = np.einsum("fhc,hc->fh", w3, a_dst)
    out = np.zeros((out_rows, pad_cols), dtype=np.float32)
    out[:f, :w.shape[1]] = wmain
    out[:f, w.shape[1]:w.shape[1] + heads] = was
    out[:f, w.shape[1] + heads:w.shape[1] + 2 * heads] = wad
    return out


_CACHE = {}


def kernel(x, edge_index, w1, att_src1, att_dst1, b1, w2, att_src2, att_dst2, b2):
    x = np.asarray(x, dtype=np.float32)
    edge_index = np.asarray(edge_index)
    src = np.concatenate([edge_index[0], np.arange(N_NODES, dtype=np.int64)]).astype(np.int64)
    dst = np.concatenate([edge_index[1], np.arange(N_NODES, dtype=np.int64)]).astype(np.int64)

    key = hash(edge_index.tobytes())
    if key not in _CACHE:
        nch, nchwb, TCH, chunk_bucket, calls, call_valid, cores = _build_layout(src, dst)
        nkern = build_kernel(nch, nchwb, TCH, chunk_bucket, call_valid)
        _CACHE[key] = (nch, TCH, cores, nkern)
    nch, TCH, cores, nkern = _CACHE[key]

    in_maps = _prep_in_maps(x, w1, att_src1, att_dst1, b1,
                            w2, att_src2, att_dst2, b2, cores)
    res = run_bass_kernel_spmd(nkern, in_maps, core_ids=list(range(R)))
    out = np.concatenate([res.results[r]["out"] for r in range(R)], axis=0)
    return out.astype(np.float32)


def _prep_in_maps(x, w1, att_src1, att_dst1, b1, w2, att_src2, att_dst2, b2, cores):
    w1p = _wext(np.asarray(w1, np.float32), np.asarray(att_src1, np.float32),
                np.asarray(att_dst1, np.float32), 128)
    w2p = _wext(np.asarray(w2, np.float32), np.asarray(att_src2, np.float32),
                np.asarray(att_dst2, np.float32), 64, permute_ch=True)
    b1rep = np.tile(np.asarray(b1, np.float32)[None, :], (128, 1))
    b2rep = np.tile(np.asarray(b2, np.float32)[None, :], (128, 1))
    iota = np.broadcast_to(np.arange(128, dtype=np.float32), (128, 128)).astype(BF16)
    ident = np.eye(128, dtype=np.float32)
    iotP = np.arange(128, dtype=np.float32).reshape(128, 1).astype(BF16)
    in_maps = []
    for r in range(R):
        xTr = np.ascontiguousarray(x[r * NLOC:(r + 1) * NLOC].T)
        in_maps.append({
            "xT": xTr, "w1p": w1p, "w2p": w2p, "b1rep": b1rep, "b2rep": b2rep,
            "iota": np.ascontiguousarray(iota), "ident": ident,
            "doff": cores[r]["dstoff"], "doffT": cores[r]["dstoffT"],
            "iotP": iotP, "pay_idx": cores[r]["pay_idx"],
        })
    return in_maps



# revision 47
# speedup vs baseline: 2.5571x; 1.8239x over previous
"""2-layer GAT on Trainium2 (8 NeuronCores), self-contained.

Sharding: edges partitioned by dst range (core r owns dst in [r*12500,(r+1)*12500)).
Per layer: node-phase matmul builds per-node tables [h | alpha_src] (+ [alpha_dst]
local), AllGather replicates the src-side table, edge phase dma_gathers per-edge
rows, computes attention with a one-hot(dst-offset) matmul trick that performs
the segment softmax denominator and the weighted aggregation in the same PSUM
accumulation. Layer 2 ends with head-mean + bias + log_softmax.
"""
import os
import numpy as np
import ml_dtypes

import concourse.bacc as bacc
import concourse.mybir as mybir
import concourse.tile as tile
from concourse.bass_utils import run_bass_kernel_spmd

BF16 = ml_dtypes.bfloat16

N_NODES = 100000
N_EDGES = 1600000
R = 8
NLOC = N_NODES // R            # 12500
WIN = 128
NWIN = (NLOC + WIN - 1) // WIN  # 98
NEG_SLOPE = 0.2
EPS = 1e-16
NBUCK = 4
SLICE_H = NLOC // NBUCK        # 3125 rows each core contributes per slice
SLAB = R * SLICE_H             # 25000-row gather region, int16-safe
CALL_MAX = 8                   # chunks per dma_gather call (1024 idxs)
SUPER = 3                      # windows per gather group
PAD_DOFF = 20000.0
TRIM = os.environ.get("K_TRIM", "0") == "1"
ZINIT = os.environ.get("K_ZINIT", "1") == "1"
GBUFS = int(os.environ.get("K_GBUFS", "3"))
SBUFS = int(os.environ.get("K_SBUFS", "2"))
ABL_NO_ADST = os.environ.get("K_NO_ADST", "0") == "1"
ABL_NO_PAY = os.environ.get("K_NO_PAY", "0") == "1"
ABL_NO_COLL = os.environ.get("K_NO_COLL", "0") == "1"
ABL_NO_AGG = os.environ.get("K_NO_AGG", "0") == "1"
ABL_NO_EVEC = os.environ.get("K_NO_EVEC", "0") == "1"


def _ceil(a, b):
    return (a + b - 1) // b


def _build_layout(src, dst):
    """Static edge layout, uniform across cores: per (window, src-bucket) cell,
    dst-sorted edges in chunks of 128 lanes. Cell-tail pad slots get idx -1 so
    the gather ucode's trailing trim skips their descriptors entirely."""
    core_of = dst // NLOC
    per_core = []
    cnt = np.zeros((R, NWIN, NBUCK), dtype=np.int64)
    for r in range(R):
        sel = np.nonzero(core_of == r)[0]
        s_ = src[sel]
        d_ = dst[sel] - r * NLOC
        w_of = d_ // WIN
        b_of = (s_ % NLOC) // SLICE_H
        order = np.lexsort((d_, b_of, w_of))
        s_, d_, w_of, b_of = s_[order], d_[order], w_of[order], b_of[order]
        np.add.at(cnt[r], (w_of, b_of), 1)
        per_core.append((s_, d_, w_of, b_of))

    nchwb = np.zeros((NWIN, NBUCK), dtype=np.int64)
    for w in range(NWIN):
        for b in range(NBUCK):
            nchwb[w, b] = max(_ceil(int(cnt[r, w, b]), WIN) for r in range(R))
    nch = nchwb.sum(axis=1)
    TCH = int(nch.sum())
    chunk_bucket = np.zeros(TCH, dtype=np.int64)
    gc = 0
    for w in range(NWIN):
        for b in range(NBUCK):
            chunk_bucket[gc:gc + nchwb[w, b]] = b
            gc += int(nchwb[w, b])

    # payload call boundaries (must mirror build_kernel): per super-group,
    # runs of same-bucket chunks, <= CALL_MAX
    calls = []   # (gc_start, n_chunks)
    gci = 0
    w = 0
    while w < NWIN:
        nwg = min(SUPER, NWIN - w)
        nch_g = int(nch[w:w + nwg].sum())
        c = 0
        while c < nch_g:
            b = chunk_bucket[gci + c]
            ce = c
            while ce < nch_g and chunk_bucket[gci + ce] == b and ce - c < CALL_MAX:
                ce += 1
            calls.append((gci + c, ce - c))
            c = ce
        gci += nch_g
        w += nwg

    cores = []
    for r in range(R):
        s_, d_, w_of, b_of = per_core[r]
        srcoff = np.full((128, TCH), -1 if TRIM else 0, dtype=np.int64)
        dstloc = np.zeros((128, TCH), dtype=np.int64)
        dstoff = np.full((128, TCH), PAD_DOFF, dtype=np.float32)
        occ = np.zeros(TCH, dtype=np.int64)
        wb = w_of * NBUCK + b_of
        bounds = np.searchsorted(wb, np.arange(NWIN * NBUCK + 1))
        gc = 0
        for w in range(NWIN):
            for b in range(NBUCK):
                lo, hi = bounds[w * NBUCK + b], bounds[w * NBUCK + b + 1]
                for ci in range(int(nchwb[w, b])):
                    e0 = lo + ci * WIN
                    k = max(0, min(WIN, hi - e0))
                    occ[gc] = k
                    if k > 0:
                        sv = s_[e0:e0 + k]
                        srcoff[:k, gc] = ((sv // NLOC) * SLICE_H
                                          + sv % NLOC - b * SLICE_H)
                        dstloc[:k, gc] = d_[e0:e0 + k]
                        dstoff[:k, gc] = d_[e0:e0 + k] - w * WIN
                    gc += 1
        # per-call valid count: trailing -1s trimmed by the ucode. Within a
        # call, pads are only in its final chunks (cell tail), so valid =
        # (sum of occ) provided every partially-filled chunk is final.
        call_valid = []
        for (g0, k) in calls:
            o = occ[g0:g0 + k]
            # prefix property check: all full before first partial
            nonfull = np.nonzero(o < WIN)[0]
            if len(nonfull):
                first = int(nonfull[0])
                assert (o[first + 1:] == 0).all(), (o, g0, k)
            call_valid.append(int(o.sum()))
        def wrap(a64, ncols):
            a = a64.astype(np.int16)
            w16 = a.T.reshape(ncols, 8, 16).transpose(2, 0, 1).reshape(16, ncols * 8)
            return np.tile(w16, (8, 1))
        dstoff_bf = dstoff.astype(BF16)
        cores.append(dict(
            pay_idx=wrap(srcoff, TCH),
            dstoff=np.ascontiguousarray(dstoff_bf),
            dstoffT=np.ascontiguousarray(dstoff_bf.T.reshape(1, TCH * 128)),
            call_valid=np.array(call_valid, dtype=np.int64),
        ))
    call_valid_u = np.stack([c["call_valid"] for c in cores], axis=0).max(axis=0)
    call_valid_u = np.minimum((call_valid_u + 15) // 16 * 16,
                              np.array([k * WIN for (_, k) in calls]))
    if TRIM:
        # mark slots beyond the uniform valid count with idx=-1 so the gather
        # ucode skips their descriptors (num_idxs_reg = non-negative count).
        for r in range(R):
            pay = cores[r]["pay_idx"]  # wrapped [128, TCH*8] int16
            for (g0, k), cv in zip(calls, call_valid_u):
                for j in range(int(cv), k * WIN):
                    ch, lane = g0 + j // WIN, j % WIN
                    pay[lane % 16::16, ch * 8 + lane // 16] = -1
    return nch, nchwb, TCH, chunk_bucket, calls, call_valid_u, cores


def build_kernel(nch, nchwb, TCH, chunk_bucket, call_valid):
    dt = mybir.dt
    f32r = dt.float32r
    nc = bacc.Bacc(None, target_bir_lowering=False, debug=True,
                   num_swdge_queues=4)

    xT = nc.dram_tensor("xT", [128, NLOC], dt.float32r, kind="ExternalInput")
    w1p = nc.dram_tensor("w1p", [128, 256], dt.float32r, kind="ExternalInput")
    w2p = nc.dram_tensor("w2p", [64, 256], dt.float32r, kind="ExternalInput")
    b1rep = nc.dram_tensor("b1rep", [128, 64], dt.float32, kind="ExternalInput")
    b2rep = nc.dram_tensor("b2rep", [128, 20], dt.float32, kind="ExternalInput")
    iota = nc.dram_tensor("iota", [128, 128], dt.bfloat16, kind="ExternalInput")
    ident = nc.dram_tensor("ident", [128, 128], dt.float32, kind="ExternalInput")
    doff_in = nc.dram_tensor("doff", [128, TCH], dt.bfloat16, kind="ExternalInput")
    doffT_in = nc.dram_tensor("doffT", [1, TCH * 128], dt.bfloat16, kind="ExternalInput")
    iotP_in = nc.dram_tensor("iotP", [128, 1], dt.bfloat16, kind="ExternalInput")
    pay_idx = nc.dram_tensor("pay_idx", [128, TCH * 8], dt.int16, kind="ExternalInput")
    out_ext = nc.dram_tensor("out", [NLOC, 20], dt.float32, kind="ExternalOutput")

    ts1_loc = nc.dram_tensor("ts1_loc", [NLOC, 128], dt.bfloat16)
    ts1_full = nc.dram_tensor("ts1_full", [N_NODES, 128], dt.bfloat16, addr_space="Shared")
    ts2_loc = nc.dram_tensor("ts2_loc", [NLOC, 256], dt.bfloat16)
    ts2_full = nc.dram_tensor("ts2_full", [N_NODES, 256], dt.bfloat16, addr_space="Shared")

    NCHMAX = int(nch.max())

    with tile.TileContext(nc) as tc:
        with (
            tc.tile_pool(name="const", bufs=1) as cpool,
            tc.tile_pool(name="sb", bufs=SBUFS) as sb,
            tc.tile_pool(name="gb", bufs=2) as gb,
            tc.tile_pool(name="ps", bufs=2, space="PSUM") as ps,
        ):
            w1t = cpool.tile([128, 256], dt.float32r, name="w1t")
            nc.sync.dma_start(out=w1t[:], in_=w1p[:])
            w2t = cpool.tile([64, 256], dt.float32r, name="w2t")
            nc.sync.dma_start(out=w2t[:], in_=w2p[:])
            b1t = cpool.tile([128, 64], dt.float32, name="b1t")
            nc.sync.dma_start(out=b1t[:], in_=b1rep[:])
            b2t = cpool.tile([128, 20], dt.float32, name="b2t")
            nc.sync.dma_start(out=b2t[:], in_=b2rep[:])
            iot = cpool.tile([128, 128], dt.bfloat16, name="iot")
            nc.sync.dma_start(out=iot[:], in_=iota[:])
            idt = cpool.tile([128, 128], dt.float32, name="idt")
            nc.sync.dma_start(out=idt[:], in_=ident[:])
            dofft = cpool.tile([128, TCH], dt.bfloat16, name="dofft")
            nc.sync.dma_start(out=dofft[:], in_=doff_in[:])
            iotP = cpool.tile([128, 1], dt.bfloat16, name="iotP")
            nc.sync.dma_start(out=iotP[:], in_=iotP_in[:])
            pidxt = cpool.tile([128, TCH * 8], dt.int16, name="pidxt")
            nc.sync.dma_start(out=pidxt[:], in_=pay_idx[:])
            # per-window [node, head] attention-dst tables, SBUF-resident
            tdT1 = cpool.tile([128, NWIN * 8], dt.bfloat16, name="tdT1")
            tdT2 = cpool.tile([128, NWIN * 8], dt.bfloat16, name="tdT2")
            nc.scalar.memzero(tdT1[:])
            nc.scalar.memzero(tdT2[:])

            # ---------- Phase A: T1 rows for local nodes ----------
            for w in range(NWIN):
                nw = min(WIN, NLOC - w * WIN)
                xt = sb.tile([128, WIN], dt.float32r, name=f"xt{w}", tag="xt")
                nc.sync.dma_start(out=xt[:, :nw], in_=xT[:, w * WIN:w * WIN + nw])
                pA = ps.tile([WIN, 256], dt.float32, name=f"pA{w}", tag="pA")
                nc.tensor.matmul(out=pA[:nw, :], lhsT=xt[:, :nw],
                                 rhs=w1t[:], start=True, stop=True)
                t1 = sb.tile([WIN, 80], dt.bfloat16, name=f"t1o{w}", tag="t1o")
                nc.vector.tensor_copy(out=t1[:nw, :], in_=pA[:nw, 0:80])
                nc.sync.dma_start(out=ts1_loc[w * WIN:w * WIN + nw, 0:72],
                                  in_=t1[:nw, 0:72])
                nc.vector.tensor_copy(out=tdT1[:nw, w * 8:w * 8 + 8],
                                      in_=t1[:nw, 72:80])

            if not ABL_NO_COLL:
                for s in range(NBUCK):
                    nc.gpsimd.collective_compute(
                        "AllGather", mybir.AluOpType.bypass,
                        replica_groups=[list(range(R))],
                        ins=[ts1_loc[s * SLICE_H:(s + 1) * SLICE_H]],
                        outs=[ts1_full[s * SLAB:(s + 1) * SLAB]])

            # ---------- window epilogues ----------
            def epilogue1(wi, nw, pagg):
                t8 = sb.tile([WIN, 8], dt.float32, name=f"t8a{wi}", tag="t8a")
                nc.vector.tensor_scalar(t8[:nw], pagg[:nw, 64:72], EPS, None,
                                        mybir.AluOpType.add)
                inv8 = sb.tile([WIN, 8], dt.float32, name=f"i8a{wi}", tag="i8a")
                nc.vector.reciprocal(inv8[:nw], t8[:nw])
                z = sb.tile([WIN, 64], dt.float32, name=f"z{wi}", tag="z")
                nc.vector.tensor_tensor(
                    out=z[:nw].rearrange("p (h e) -> p h e", e=8),
                    in0=pagg[:nw, 0:64].rearrange("p (h e) -> p h e", e=8),
                    in1=inv8[:nw].unsqueeze(2).broadcast_to([nw, 8, 8]),
                    op=mybir.AluOpType.mult)
                nc.vector.tensor_tensor(out=z[:nw], in0=z[:nw], in1=b1t[:nw],
                                        op=mybir.AluOpType.add)
                mz = sb.tile([WIN, 64], dt.float32, name=f"mz{wi}", tag="mz")
                nc.vector.tensor_scalar(mz[:nw], z[:nw], 0.0, None,
                                        mybir.AluOpType.min)
                nc.scalar.activation(mz[:nw], mz[:nw],
                                     mybir.ActivationFunctionType.Exp)
                rz = sb.tile([WIN, 64], dt.float32, name=f"rz{wi}", tag="rz")
                nc.scalar.activation(rz[:nw], z[:nw],
                                     mybir.ActivationFunctionType.Relu)
                nc.vector.tensor_tensor(out=z[:nw], in0=mz[:nw], in1=rz[:nw],
                                        op=mybir.AluOpType.add)
                h2 = sb.tile([WIN, 64], dt.float32, name=f"h2{wi}", tag="h2")
                nc.vector.tensor_scalar(h2[:nw], z[:nw], -1.0, None,
                                        mybir.AluOpType.add)
                ptr = ps.tile([64, WIN], dt.float32, name=f"ptr{wi}", tag="ptr")
                nc.tensor.transpose(out=ptr[:, :nw], in_=h2[:nw, :], identity=idt[:nw, :nw])
                h2T = sb.tile([64, WIN], dt.float32r, name=f"h2T{wi}", tag="h2T")
                nc.scalar.copy(out=h2T[:, :nw], in_=ptr[:, :nw])
                pT2 = ps.tile([WIN, 256], dt.float32, name=f"pT2{wi}", tag="pA")
                nc.tensor.matmul(out=pT2[:nw, :], lhsT=h2T[:, :nw],
                                 rhs=w2t[:], start=True, stop=True)
                t2o = sb.tile([WIN, 176], dt.bfloat16, name=f"t2o{wi}", tag="t2o")
                nc.vector.tensor_copy(out=t2o[:nw, :], in_=pT2[:nw, 0:176])
                nc.sync.dma_start(out=ts2_loc[wi * WIN:wi * WIN + nw, 0:168],
                                  in_=t2o[:nw, 0:168])
                nc.vector.tensor_copy(out=tdT2[:nw, wi * 8:wi * 8 + 8],
                                      in_=t2o[:nw, 168:176])

            def epilogue2(wi, nw, pagg):
                t8 = sb.tile([WIN, 8], dt.float32, name=f"t8b{wi}", tag="t8b")
                nc.vector.tensor_scalar(t8[:nw], pagg[:nw, 160:168], EPS, 8.0,
                                        mybir.AluOpType.add, mybir.AluOpType.mult)
                inv8 = sb.tile([WIN, 8], dt.float32, name=f"i8b{wi}", tag="i8b")
                nc.vector.reciprocal(inv8[:nw], t8[:nw])
                v = sb.tile([WIN, 20], dt.float32, name=f"v{wi}", tag="v")
                val = sb.tile([WIN, 160], dt.float32, name=f"val{wi}", tag="val")
                nc.vector.tensor_tensor(
                    out=val[:nw].rearrange("p (c h) -> p c h", h=8),
                    in0=pagg[:nw, 0:160].rearrange("p (c h) -> p c h", h=8),
                    in1=inv8[:nw].unsqueeze(1).broadcast_to([nw, 20, 8]),
                    op=mybir.AluOpType.mult)
                nc.vector.tensor_reduce(
                    out=v[:nw], in_=val[:nw].rearrange("p (c h) -> p c h", h=8),
                    axis=mybir.AxisListType.X, op=mybir.AluOpType.add)
                nc.vector.tensor_tensor(out=v[:nw], in0=v[:nw], in1=b2t[:nw],
                                        op=mybir.AluOpType.add)
                mx = sb.tile([WIN, 1], dt.float32, name=f"mx{wi}", tag="mx")
                nc.vector.tensor_reduce(out=mx[:nw], in_=v[:nw],
                                        axis=mybir.AxisListType.X,
                                        op=mybir.AluOpType.max)
                nmx = sb.tile([WIN, 1], dt.float32, name=f"nmx{wi}", tag="nmx")
                nc.vector.tensor_scalar(nmx[:nw], mx[:nw], -1.0, None,
                                        mybir.AluOpType.mult)
                ex = sb.tile([WIN, 20], dt.float32, name=f"exo{wi}", tag="exo")
                ssum = sb.tile([WIN, 1], dt.float32, name=f"ss{wi}", tag="ss")
                nc.scalar.activation(ex[:nw], v[:nw],
                                     mybir.ActivationFunctionType.Exp,
                                     bias=nmx[:nw], accum_out=ssum[:nw])
                lse = sb.tile([WIN, 1], dt.float32, name=f"lse{wi}", tag="lse")
                nc.scalar.activation(lse[:nw], ssum[:nw],
                                     mybir.ActivationFunctionType.Ln)
                nc.vector.tensor_tensor(
                    out=ex[:nw], in0=v[:nw],
                    in1=nmx[:nw].broadcast_to([nw, 20]),
                    op=mybir.AluOpType.add)
                ot = sb.tile([WIN, 20], dt.float32, name=f"ot{wi}", tag="ot")
                nc.vector.tensor_tensor(
                    out=ot[:nw], in0=ex[:nw],
                    in1=lse[:nw].broadcast_to([nw, 20]),
                    op=mybir.AluOpType.subtract)
                nc.sync.dma_start(out=out_ext[wi * WIN:wi * WIN + nw, :],
                                  in_=ot[:nw, :])

            # ---------- slot init: zero the two gbuf pool slots so lanes
            # skipped by the gather trim read finite stale data ----------
            if ZINIT:
                for ii in range(GBUFS):
                    gz = gb.tile([128, NCHMAX * SUPER, 256], dt.bfloat16,
                                 name=f"gz{ii}", tag="gbuf", bufs=GBUFS)
                    nc.scalar.memzero(gz[:])

            # ---------- edge phase ----------

            def edge_phase(layer, slice_done=None):
                call_counter = [0]
                # last window whose epilogue completes collective slice s
                slice_last_w = [(min((s + 1) * SLICE_H, NLOC) - 1) // WIN
                                for s in range(NBUCK)]
                if layer == 1:
                    tsrc_full, tdT = ts1_full, tdT1
                    EW, CY, CH, NH_IN = 128, 72, 64, 8
                else:
                    tsrc_full, tdT = ts2_full, tdT2
                    EW, CY, CH, NH_IN = 256, 168, 160, 20

                gc0 = 0
                qn = 0
                w = 0
                while w < NWIN:
                    nwg = min(SUPER, NWIN - w)
                    nch_g = int(nch[w:w + nwg].sum())
                    gbuf = gb.tile([128, nch_g, EW], dt.bfloat16,
                                   name=f"gbuf{layer}_{w}", tag="gbuf", bufs=GBUFS,
                                   padded_shape=[128, NCHMAX * SUPER * (256 // EW), EW])

                    c = 0
                    while c < nch_g and not ABL_NO_PAY:
                        b = int(chunk_bucket[gc0 + c])
                        ce = c
                        while (ce < nch_g and int(chunk_bucket[gc0 + ce]) == b
                               and ce - c < CALL_MAX):
                            ce += 1
                        k = ce - c
                        cv = int(call_valid[call_counter[0]]) if TRIM else k * 128
                        call_counter[0] += 1
                        nc.gpsimd.dma_gather(
                            gbuf[:, c:ce, :],
                            tsrc_full[b * SLAB:(b + 1) * SLAB, :],
                            pidxt[:, (gc0 + c) * 8:(gc0 + ce) * 8],
                            k * 128, cv, EW,
                            queue_num=qn % 4, single_packet=True)
                        qn += 1
                        c = ce
                    cw = 0
                    for wi in range(w, w + nwg):
                        nchw = int(nch[wi])
                        nw_nodes = min(WIN, NLOC - wi * WIN)
                        sl = slice(cw, cw + nchw)
                        # transposed one-hot [node, (chunk, lane)] for the
                        # per-edge adst matmul
                        dfT = sb.tile([128, nchw * 128], dt.bfloat16,
                                      name=f"dfT{layer}_{wi}", tag="dfT",
                                      padded_shape=[128, NCHMAX * 128])
                        nc.sync.dma_start(
                            out=dfT[:],
                            in_=doffT_in[0:1, (gc0 + cw) * 128:(gc0 + cw + nchw) * 128]
                                .broadcast_to([128, nchw * 128]))
                        ohT = sb.tile([128, nchw * 128], dt.bfloat16,
                                      name=f"ohT{layer}_{wi}", tag="ohT",
                                      padded_shape=[128, NCHMAX * 128])
                        nc.vector.tensor_tensor(
                            out=ohT[:],
                            in0=iotP[:, 0:1].broadcast_to([128, nchw * 128]),
                            in1=dfT[:], op=mybir.AluOpType.is_equal)
                        eadst = ps.tile([128, nchw * 8], dt.float32,
                                        name=f"ea{layer}_{wi}", tag="eadst",
                                        padded_shape=[128, NCHMAX * 8])
                        for ci in range(nchw):
                            nc.tensor.matmul(
                                out=eadst[:, ci * 8:ci * 8 + 8],
                                lhsT=ohT[:, ci * 128:ci * 128 + 128],
                                rhs=tdT[:, wi * 8:wi * 8 + 8],
                                start=True, stop=True)
                        ebuf = sb.tile([128, nchw, 8], dt.float32,
                                       name=f"eb{layer}_{wi}", tag="ebuf",
                                       padded_shape=[128, NCHMAX, 8])
                        nc.vector.tensor_tensor(
                            out=ebuf[:], in0=gbuf[:, sl, CH:CH + 8],
                            in1=eadst[:].rearrange("p (c h) -> p c h", h=8),
                            op=mybir.AluOpType.add)
                        nc.scalar.activation(ebuf[:], ebuf[:],
                                             mybir.ActivationFunctionType.Prelu,
                                             alpha=NEG_SLOPE)
                        nc.scalar.activation(gbuf[:, sl, CH:CH + 8], ebuf[:],
                                             mybir.ActivationFunctionType.Exp)
                        if layer == 1:
                            nc.vector.tensor_tensor(
                                out=gbuf[:, sl, 0:CH].rearrange(
                                    "p c (h e) -> p c h e", e=8),
                                in0=gbuf[:, sl, 0:CH].rearrange(
                                    "p c (h e) -> p c h e", e=8),
                                in1=gbuf[:, sl, CH:CH + 8].unsqueeze(3)
                                    .broadcast_to([128, nchw, 8, 8]),
                                op=mybir.AluOpType.mult)
                        else:
                            nc.vector.tensor_tensor(
                                out=gbuf[:, sl, 0:CH].rearrange(
                                    "p c (e h) -> p c e h", h=8),
                                in0=gbuf[:, sl, 0:CH].rearrange(
                                    "p c (e h) -> p c e h", h=8),
                                in1=gbuf[:, sl, CH:CH + 8].unsqueeze(2)
                                    .broadcast_to([128, nchw, 20, 8]),
                                op=mybir.AluOpType.mult)
                        oh = sb.tile([128, nchw, 128], dt.bfloat16,
                                     name=f"oh{layer}_{wi}", tag="oh",
                                     padded_shape=[128, NCHMAX, 128])
                        nc.vector.tensor_tensor(
                            out=oh[:],
                            in0=dofft[:, gc0 + cw:gc0 + cw + nchw]
                                .unsqueeze(2).broadcast_to([128, nchw, 128]),
                            in1=iot[:].unsqueeze(1).broadcast_to([128, nchw, 128]),
                            op=mybir.AluOpType.is_equal)
                        pagg = ps.tile([WIN, CY], dt.float32,
                                       name=f"pg{layer}_{wi}", tag="pagg",
                                       padded_shape=[WIN, 168])
                        nmm = 1 if ABL_NO_AGG else nchw
                        for ci in range(nmm):
                            nc.tensor.matmul(
                                out=pagg[:, :],
                                lhsT=oh[:, ci, :],
                                rhs=gbuf[:, cw + ci, 0:CY],
                                start=(ci == 0), stop=(ci == nmm - 1))
                        if layer == 1:
                            epilogue1(wi, nw_nodes, pagg)
                        else:
                            epilogue2(wi, nw_nodes, pagg)
                        if slice_done is not None and wi in slice_last_w:
                            slice_done(slice_last_w.index(wi))
                        cw += nchw
                    gc0 += nch_g
                    w += nwg

            def coll2_slice(s):
                if not ABL_NO_COLL:
                    nc.gpsimd.collective_compute(
                        "AllGather", mybir.AluOpType.bypass,
                        replica_groups=[list(range(R))],
                        ins=[ts2_loc[s * SLICE_H:(s + 1) * SLICE_H]],
                        outs=[ts2_full[s * SLAB:(s + 1) * SLAB]])

            edge_phase(1, coll2_slice)
            edge_phase(2)

    nc.compile()
    return nc


def _wext(w, a_src, a_dst, out_rows, pad_cols=256, permute_ch=False, heads=8):
    """[W | W.a_src | W.a_dst] padded to [out_rows, pad_cols] f32."""
    f = w.shape[0]
    c = w.shape[1] // heads
    w3 = w.reshape(f, heads, c)
    if permute_ch:
        wmain = w3.transpose(0, 2, 1).reshape(f, heads * c)  # col = ch*H + head
    else:
        wmain = w
    was = np.einsum("fhc,hc->fh", w3, a_src)
    wad = np.einsum("fhc,hc->fh", w3, a_dst)
    out = np.zeros((out_rows, pad_cols), dtype=np.float32)
    out[:f, :w.shape[1]] = wmain
    out[:f, w.shape[1]:w.shape[1] + heads] = was
    out[:f, w.shape[1] + heads:w.shape[1] + 2 * heads] = wad
    return out


_CACHE = {}


def kernel(x, edge_index, w1, att_src1, att_dst1, b1, w2, att_src2, att_dst2, b2):
    x = np.asarray(x, dtype=np.float32)
    edge_index = np.asarray(edge_index)
    src = np.concatenate([edge_index[0], np.arange(N_NODES, dtype=np.int64)]).astype(np.int64)
    dst = np.concatenate([edge_index[1], np.arange(N_NODES, dtype=np.int64)]).astype(np.int64)

    key = hash(edge_index.tobytes())
    if key not in _CACHE:
        nch, nchwb, TCH, chunk_bucket, calls, call_valid, cores = _build_layout(src, dst)
        nkern = build_kernel(nch, nchwb, TCH, chunk_bucket, call_valid)
        _CACHE[key] = (nch, TCH, cores, nkern)
    nch, TCH, cores, nkern = _CACHE[key]

    in_maps = _prep_in_maps(x, w1, att_src1, att_dst1, b1,
                            w2, att_src2, att_dst2, b2, cores)
    res = run_bass_kernel_spmd(nkern, in_maps, core_ids=list(range(R)))
    out = np.concatenate([res.results[r]["out"] for r in range(R)], axis=0)
    return out.astype(np.float32)


def _prep_in_maps(x, w1, att_src1, att_dst1, b1, w2, att_src2, att_dst2, b2, cores):
    w1p = _wext(np.asarray(w1, np.float32), np.asarray(att_src1, np.float32),
                np.asarray(att_dst1, np.float32), 128)
    w2p = _wext(np.asarray(w2, np.float32), np.asarray(att_src2, np.float32),
                np.asarray(att_dst2, np.float32), 64, permute_ch=True)
    b1rep = np.tile(np.asarray(b1, np.float32)[None, :], (128, 1))
    b2rep = np.tile(np.asarray(b2, np.float32)[None, :], (128, 1))
    iota = np.broadcast_to(np.arange(128, dtype=np.float32), (128, 128)).astype(BF16)
    ident = np.eye(128, dtype=np.float32)
    iotP = np.arange(128, dtype=np.float32).reshape(128, 1).astype(BF16)
    in_maps = []
    for r in range(R):
        xTr = np.ascontiguousarray(x[r * NLOC:(r + 1) * NLOC].T)
        in_maps.append({
            "xT": xTr, "w1p": w1p, "w2p": w2p, "b1rep": b1rep, "b2rep": b2rep,
            "iota": np.ascontiguousarray(iota), "ident": ident,
            "doff": cores[r]["dstoff"], "doffT": cores[r]["dstoffT"],
            "iotP": iotP, "pay_idx": cores[r]["pay_idx"],
        })
    return in_maps



# revision 54
# speedup vs baseline: 6.3121x; 2.4685x over previous
"""2-layer GAT on Trainium2 (8 NeuronCores), self-contained.

Sharding: edges partitioned by dst range (core r owns dst in [r*12500,(r+1)*12500)).
Per layer: node-phase matmul builds per-node tables [h | alpha_src] (+ [alpha_dst]
local), AllGather replicates the src-side table, edge phase dma_gathers per-edge
rows, computes attention with a one-hot(dst-offset) matmul trick that performs
the segment softmax denominator and the weighted aggregation in the same PSUM
accumulation. Layer 2 ends with head-mean + bias + log_softmax.
"""
import os
import numpy as np
import ml_dtypes

import concourse.bacc as bacc
import concourse.mybir as mybir
import concourse.tile as tile
from concourse.bass_utils import run_bass_kernel_spmd

BF16 = ml_dtypes.bfloat16

N_NODES = 100000
N_EDGES = 1600000
R = 8
NLOC = N_NODES // R            # 12500
WIN = 128
NWIN = (NLOC + WIN - 1) // WIN  # 98
NEG_SLOPE = 0.2
EPS = 1e-16
NBUCK = 4
SLICE_H = NLOC // NBUCK        # 3125 rows each core contributes per slice
SLAB = R * SLICE_H             # 25000-row gather region, int16-safe
CALL_MAX = 8                   # chunks per dma_gather call (1024 idxs)
SUPER = 3                      # windows per gather group
PAD_DOFF = 20000.0
TRIM = os.environ.get("K_TRIM", "0") == "1"
ZINIT = os.environ.get("K_ZINIT", "1") == "1"
GBUFS = int(os.environ.get("K_GBUFS", "2"))
SBUFS = int(os.environ.get("K_SBUFS", "3"))
ABL_NO_ADST = os.environ.get("K_NO_ADST", "0") == "1"
ABL_NO_PAY = os.environ.get("K_NO_PAY", "0") == "1"
ABL_NO_COLL = os.environ.get("K_NO_COLL", "0") == "1"
ABL_NO_AGG = os.environ.get("K_NO_AGG", "0") == "1"
ABL_NO_EVEC = os.environ.get("K_NO_EVEC", "0") == "1"


def _ceil(a, b):
    return (a + b - 1) // b


def _build_layout(src, dst):
    """Static edge layout, uniform across cores: per (window, src-bucket) cell,
    dst-sorted edges in chunks of 128 lanes. Cell-tail pad slots get idx -1 so
    the gather ucode's trailing trim skips their descriptors entirely."""
    core_of = dst // NLOC
    per_core = []
    cnt = np.zeros((R, NWIN, NBUCK), dtype=np.int64)
    for r in range(R):
        sel = np.nonzero(core_of == r)[0]
        s_ = src[sel]
        d_ = dst[sel] - r * NLOC
        w_of = d_ // WIN
        b_of = (s_ % NLOC) // SLICE_H
        order = np.lexsort((d_, b_of, w_of))
        s_, d_, w_of, b_of = s_[order], d_[order], w_of[order], b_of[order]
        np.add.at(cnt[r], (w_of, b_of), 1)
        per_core.append((s_, d_, w_of, b_of))

    nchwb = np.zeros((NWIN, NBUCK), dtype=np.int64)
    for w in range(NWIN):
        for b in range(NBUCK):
            nchwb[w, b] = max(_ceil(int(cnt[r, w, b]), WIN) for r in range(R))
    nch = nchwb.sum(axis=1)
    TCH = int(nch.sum())
    chunk_bucket = np.zeros(TCH, dtype=np.int64)
    gc = 0
    for w in range(NWIN):
        for b in range(NBUCK):
            chunk_bucket[gc:gc + nchwb[w, b]] = b
            gc += int(nchwb[w, b])

    # payload call boundaries (must mirror build_kernel): per super-group,
    # runs of same-bucket chunks, <= CALL_MAX
    calls = []   # (gc_start, n_chunks)
    gci = 0
    w = 0
    while w < NWIN:
        nwg = min(SUPER, NWIN - w)
        nch_g = int(nch[w:w + nwg].sum())
        c = 0
        while c < nch_g:
            b = chunk_bucket[gci + c]
            ce = c
            while ce < nch_g and chunk_bucket[gci + ce] == b and ce - c < CALL_MAX:
                ce += 1
            calls.append((gci + c, ce - c))
            c = ce
        gci += nch_g
        w += nwg

    cores = []
    for r in range(R):
        s_, d_, w_of, b_of = per_core[r]
        if TRIM:
            srcoff = np.full((128, TCH), -1, dtype=np.int64)
        else:
            # scatter pad-slot reads across rows: same-row gathers serialize
            # on one HBM bank
            srcoff = (np.arange(128 * TCH, dtype=np.int64).reshape(TCH, 128).T
                      * 37) % SLAB
        dstloc = np.zeros((128, TCH), dtype=np.int64)
        dstoff = np.full((128, TCH), PAD_DOFF, dtype=np.float32)
        occ = np.zeros(TCH, dtype=np.int64)
        wb = w_of * NBUCK + b_of
        bounds = np.searchsorted(wb, np.arange(NWIN * NBUCK + 1))
        gc = 0
        for w in range(NWIN):
            for b in range(NBUCK):
                lo, hi = bounds[w * NBUCK + b], bounds[w * NBUCK + b + 1]
                for ci in range(int(nchwb[w, b])):
                    e0 = lo + ci * WIN
                    k = max(0, min(WIN, hi - e0))
                    occ[gc] = k
                    if k > 0:
                        sv = s_[e0:e0 + k]
                        srcoff[:k, gc] = ((sv // NLOC) * SLICE_H
                                          + sv % NLOC - b * SLICE_H)
                        dstloc[:k, gc] = d_[e0:e0 + k]
                        dstoff[:k, gc] = d_[e0:e0 + k] - w * WIN
                    gc += 1
        # per-call valid count: trailing -1s trimmed by the ucode. Within a
        # call, pads are only in its final chunks (cell tail), so valid =
        # (sum of occ) provided every partially-filled chunk is final.
        call_valid = []
        for (g0, k) in calls:
            o = occ[g0:g0 + k]
            # prefix property check: all full before first partial
            nonfull = np.nonzero(o < WIN)[0]
            if len(nonfull):
                first = int(nonfull[0])
                assert (o[first + 1:] == 0).all(), (o, g0, k)
            call_valid.append(int(o.sum()))
        def wrap(a64, ncols):
            a = a64.astype(np.int16)
            w16 = a.T.reshape(ncols, 8, 16).transpose(2, 0, 1).reshape(16, ncols * 8)
            return np.tile(w16, (8, 1))
        dstoff_bf = dstoff.astype(BF16)
        dstoff_i8 = np.where(dstoff == PAD_DOFF, -1, dstoff).astype(np.int8)
        cores.append(dict(
            pay_idx=wrap(srcoff, TCH),
            dstoff=np.ascontiguousarray(dstoff_bf),
            dstoffT=np.ascontiguousarray(dstoff_i8.T.reshape(1, TCH * 128)),
            call_valid=np.array(call_valid, dtype=np.int64),
        ))
    call_valid_u = np.stack([c["call_valid"] for c in cores], axis=0).max(axis=0)
    call_valid_u = np.minimum((call_valid_u + 15) // 16 * 16,
                              np.array([k * WIN for (_, k) in calls]))
    if TRIM:
        # mark slots beyond the uniform valid count with idx=-1 so the gather
        # ucode skips their descriptors (num_idxs_reg = non-negative count).
        for r in range(R):
            pay = cores[r]["pay_idx"]  # wrapped [128, TCH*8] int16
            for (g0, k), cv in zip(calls, call_valid_u):
                for j in range(int(cv), k * WIN):
                    ch, lane = g0 + j // WIN, j % WIN
                    pay[lane % 16::16, ch * 8 + lane // 16] = -1
    return nch, nchwb, TCH, chunk_bucket, calls, call_valid_u, cores


def build_kernel(nch, nchwb, TCH, chunk_bucket, call_valid):
    dt = mybir.dt
    f32r = dt.float32r
    nc = bacc.Bacc(None, target_bir_lowering=False, debug=True,
                   num_swdge_queues=4)

    xT = nc.dram_tensor("xT", [128, NLOC], dt.float32r, kind="ExternalInput")
    w1p = nc.dram_tensor("w1p", [128, 256], dt.float32r, kind="ExternalInput")
    w2p = nc.dram_tensor("w2p", [64, 256], dt.float32r, kind="ExternalInput")
    b1rep = nc.dram_tensor("b1rep", [128, 64], dt.float32, kind="ExternalInput")
    b2rep = nc.dram_tensor("b2rep", [128, 20], dt.float32, kind="ExternalInput")
    iota = nc.dram_tensor("iota", [128, 128], dt.bfloat16, kind="ExternalInput")
    ident = nc.dram_tensor("ident", [128, 128], dt.float32, kind="ExternalInput")
    doff_in = nc.dram_tensor("doff", [128, TCH], dt.bfloat16, kind="ExternalInput")
    doffT_in = nc.dram_tensor("doffT", [1, TCH * 128], dt.int8, kind="ExternalInput")
    iotP_in = nc.dram_tensor("iotP", [128, 1], dt.int8, kind="ExternalInput")
    pay_idx = nc.dram_tensor("pay_idx", [128, TCH * 8], dt.int16, kind="ExternalInput")
    out_ext = nc.dram_tensor("out", [NLOC, 20], dt.float32, kind="ExternalOutput")

    ts1_loc = nc.dram_tensor("ts1_loc", [NLOC, 128], dt.bfloat16)
    ts1_full = nc.dram_tensor("ts1_full", [N_NODES, 128], dt.bfloat16, addr_space="Shared")
    ts2_loc = nc.dram_tensor("ts2_loc", [NLOC, 256], dt.bfloat16)
    ts2_full = nc.dram_tensor("ts2_full", [N_NODES, 256], dt.bfloat16, addr_space="Shared")

    NCHMAX = int(nch.max())

    with tile.TileContext(nc) as tc:
        with (
            tc.tile_pool(name="const", bufs=1) as cpool,
            tc.tile_pool(name="sb", bufs=SBUFS) as sb,
            tc.tile_pool(name="gb", bufs=2) as gb,
            tc.tile_pool(name="ps", bufs=2, space="PSUM") as ps,
        ):
            w1t = cpool.tile([128, 256], dt.float32r, name="w1t")
            nc.sync.dma_start(out=w1t[:], in_=w1p[:])
            w2t = cpool.tile([64, 256], dt.float32r, name="w2t")
            nc.sync.dma_start(out=w2t[:], in_=w2p[:])
            b1t = cpool.tile([128, 64], dt.float32, name="b1t")
            nc.sync.dma_start(out=b1t[:], in_=b1rep[:])
            b2t = cpool.tile([128, 20], dt.float32, name="b2t")
            nc.sync.dma_start(out=b2t[:], in_=b2rep[:])
            iot = cpool.tile([128, 128], dt.bfloat16, name="iot")
            nc.sync.dma_start(out=iot[:], in_=iota[:])
            idt = cpool.tile([128, 128], dt.float32, name="idt")
            nc.sync.dma_start(out=idt[:], in_=ident[:])
            dofft = cpool.tile([128, TCH], dt.bfloat16, name="dofft")
            nc.sync.dma_start(out=dofft[:], in_=doff_in[:])
            iotP = cpool.tile([128, 1], dt.int8, name="iotP")
            nc.sync.dma_start(out=iotP[:], in_=iotP_in[:])
            pidxt = cpool.tile([128, TCH * 8], dt.int16, name="pidxt")
            nc.sync.dma_start(out=pidxt[:], in_=pay_idx[:])
            # per-window [node, head] attention-dst tables, SBUF-resident
            tdT1 = cpool.tile([128, NWIN * 8], dt.bfloat16, name="tdT1")
            tdT2 = cpool.tile([128, NWIN * 8], dt.bfloat16, name="tdT2")
            nc.scalar.memzero(tdT1[:])
            nc.scalar.memzero(tdT2[:])

            # ---------- Phase A: T1 rows for local nodes ----------
            for w in range(NWIN):
                nw = min(WIN, NLOC - w * WIN)
                xt = sb.tile([128, WIN], dt.float32r, name=f"xt{w}", tag="xt")
                nc.sync.dma_start(out=xt[:, :nw], in_=xT[:, w * WIN:w * WIN + nw])
                pA = ps.tile([WIN, 256], dt.float32, name=f"pA{w}", tag="pA")
                nc.tensor.matmul(out=pA[:nw, :], lhsT=xt[:, :nw],
                                 rhs=w1t[:], start=True, stop=True)
                t1 = sb.tile([WIN, 80], dt.bfloat16, name=f"t1o{w}", tag="t1o")
                nc.vector.tensor_copy(out=t1[:nw, :], in_=pA[:nw, 0:80])
                nc.sync.dma_start(out=ts1_loc[w * WIN:w * WIN + nw, 0:72],
                                  in_=t1[:nw, 0:72])
                nc.vector.tensor_copy(out=tdT1[:nw, w * 8:w * 8 + 8],
                                      in_=t1[:nw, 72:80])

            if not ABL_NO_COLL:
                for s in range(NBUCK):
                    nc.gpsimd.collective_compute(
                        "AllGather", mybir.AluOpType.bypass,
                        replica_groups=[list(range(R))],
                        ins=[ts1_loc[s * SLICE_H:(s + 1) * SLICE_H]],
                        outs=[ts1_full[s * SLAB:(s + 1) * SLAB]])

            # ---------- window epilogues ----------
            def epilogue1(wi, nw, pagg):
                t8 = sb.tile([WIN, 8], dt.float32, name=f"t8a{wi}", tag="t8a")
                nc.vector.tensor_scalar(t8[:nw], pagg[:nw, 64:72], EPS, None,
                                        mybir.AluOpType.add)
                inv8 = sb.tile([WIN, 8], dt.float32, name=f"i8a{wi}", tag="i8a")
                nc.vector.reciprocal(inv8[:nw], t8[:nw])
                z = sb.tile([WIN, 64], dt.float32, name=f"z{wi}", tag="z")
                nc.vector.tensor_tensor(
                    out=z[:nw].rearrange("p (h e) -> p h e", e=8),
                    in0=pagg[:nw, 0:64].rearrange("p (h e) -> p h e", e=8),
                    in1=inv8[:nw].unsqueeze(2).broadcast_to([nw, 8, 8]),
                    op=mybir.AluOpType.mult)
                nc.vector.tensor_tensor(out=z[:nw], in0=z[:nw], in1=b1t[:nw],
                                        op=mybir.AluOpType.add)
                mz = sb.tile([WIN, 64], dt.float32, name=f"mz{wi}", tag="mz")
                nc.vector.tensor_scalar(mz[:nw], z[:nw], 0.0, None,
                                        mybir.AluOpType.min)
                nc.scalar.activation(mz[:nw], mz[:nw],
                                     mybir.ActivationFunctionType.Exp)
                rz = sb.tile([WIN, 64], dt.float32, name=f"rz{wi}", tag="rz")
                nc.scalar.activation(rz[:nw], z[:nw],
                                     mybir.ActivationFunctionType.Relu)
                nc.vector.tensor_tensor(out=z[:nw], in0=mz[:nw], in1=rz[:nw],
                                        op=mybir.AluOpType.add)
                h2 = sb.tile([WIN, 64], dt.float32, name=f"h2{wi}", tag="h2")
                nc.vector.tensor_scalar(h2[:nw], z[:nw], -1.0, None,
                                        mybir.AluOpType.add)
                ptr = ps.tile([64, WIN], dt.float32, name=f"ptr{wi}", tag="ptr")
                nc.tensor.transpose(out=ptr[:, :nw], in_=h2[:nw, :], identity=idt[:nw, :nw])
                h2T = sb.tile([64, WIN], dt.float32r, name=f"h2T{wi}", tag="h2T")
                nc.scalar.copy(out=h2T[:, :nw], in_=ptr[:, :nw])
                pT2 = ps.tile([WIN, 256], dt.float32, name=f"pT2{wi}", tag="pA")
                nc.tensor.matmul(out=pT2[:nw, :], lhsT=h2T[:, :nw],
                                 rhs=w2t[:], start=True, stop=True)
                t2o = sb.tile([WIN, 176], dt.bfloat16, name=f"t2o{wi}", tag="t2o")
                nc.vector.tensor_copy(out=t2o[:nw, :], in_=pT2[:nw, 0:176])
                nc.sync.dma_start(out=ts2_loc[wi * WIN:wi * WIN + nw, 0:168],
                                  in_=t2o[:nw, 0:168])
                nc.vector.tensor_copy(out=tdT2[:nw, wi * 8:wi * 8 + 8],
                                      in_=t2o[:nw, 168:176])

            def epilogue2(wi, nw, pagg):
                t8 = sb.tile([WIN, 8], dt.float32, name=f"t8b{wi}", tag="t8b")
                nc.vector.tensor_scalar(t8[:nw], pagg[:nw, 160:168], EPS, 8.0,
                                        mybir.AluOpType.add, mybir.AluOpType.mult)
                inv8 = sb.tile([WIN, 8], dt.float32, name=f"i8b{wi}", tag="i8b")
                nc.vector.reciprocal(inv8[:nw], t8[:nw])
                v = sb.tile([WIN, 20], dt.float32, name=f"v{wi}", tag="v")
                val = sb.tile([WIN, 160], dt.float32, name=f"val{wi}", tag="val")
                nc.vector.tensor_tensor(
                    out=val[:nw].rearrange("p (c h) -> p c h", h=8),
                    in0=pagg[:nw, 0:160].rearrange("p (c h) -> p c h", h=8),
                    in1=inv8[:nw].unsqueeze(1).broadcast_to([nw, 20, 8]),
                    op=mybir.AluOpType.mult)
                nc.vector.tensor_reduce(
                    out=v[:nw], in_=val[:nw].rearrange("p (c h) -> p c h", h=8),
                    axis=mybir.AxisListType.X, op=mybir.AluOpType.add)
                nc.vector.tensor_tensor(out=v[:nw], in0=v[:nw], in1=b2t[:nw],
                                        op=mybir.AluOpType.add)
                mx = sb.tile([WIN, 1], dt.float32, name=f"mx{wi}", tag="mx")
                nc.vector.tensor_reduce(out=mx[:nw], in_=v[:nw],
                                        axis=mybir.AxisListType.X,
                                        op=mybir.AluOpType.max)
                nmx = sb.tile([WIN, 1], dt.float32, name=f"nmx{wi}", tag="nmx")
                nc.vector.tensor_scalar(nmx[:nw], mx[:nw], -1.0, None,
                                        mybir.AluOpType.mult)
                ex = sb.tile([WIN, 20], dt.float32, name=f"exo{wi}", tag="exo")
                ssum = sb.tile([WIN, 1], dt.float32, name=f"ss{wi}", tag="ss")
                nc.scalar.activation(ex[:nw], v[:nw],
                                     mybir.ActivationFunctionType.Exp,
                                     bias=nmx[:nw], accum_out=ssum[:nw])
                lse = sb.tile([WIN, 1], dt.float32, name=f"lse{wi}", tag="lse")
                nc.scalar.activation(lse[:nw], ssum[:nw],
                                     mybir.ActivationFunctionType.Ln)
                nc.vector.tensor_tensor(
                    out=ex[:nw], in0=v[:nw],
                    in1=nmx[:nw].broadcast_to([nw, 20]),
                    op=mybir.AluOpType.add)
                ot = sb.tile([WIN, 20], dt.float32, name=f"ot{wi}", tag="ot")
                nc.vector.tensor_tensor(
                    out=ot[:nw], in0=ex[:nw],
                    in1=lse[:nw].broadcast_to([nw, 20]),
                    op=mybir.AluOpType.subtract)
                nc.sync.dma_start(out=out_ext[wi * WIN:wi * WIN + nw, :],
                                  in_=ot[:nw, :])

            # ---------- slot init: zero the two gbuf pool slots so lanes
            # skipped by the gather trim read finite stale data ----------
            if ZINIT:
                for ii in range(GBUFS):
                    gz = gb.tile([128, NCHMAX * SUPER, 256], dt.bfloat16,
                                 name=f"gz{ii}", tag="gbuf", bufs=GBUFS)
                    nc.scalar.memzero(gz[:])

            # ---------- edge phase ----------

            def edge_phase(layer, slice_done=None):
                call_counter = [0]
                # last window whose epilogue completes collective slice s
                slice_last_w = [(min((s + 1) * SLICE_H, NLOC) - 1) // WIN
                                for s in range(NBUCK)]
                if layer == 1:
                    tsrc_full, tdT = ts1_full, tdT1
                    EW, CY, CH, NH_IN = 128, 72, 64, 8
                else:
                    tsrc_full, tdT = ts2_full, tdT2
                    EW, CY, CH, NH_IN = 256, 168, 160, 20

                gc0 = 0
                qn = 0
                w = 0
                while w < NWIN:
                    nwg = min(SUPER, NWIN - w)
                    nch_g = int(nch[w:w + nwg].sum())
                    gbuf = gb.tile([128, nch_g, EW], dt.bfloat16,
                                   name=f"gbuf{layer}_{w}", tag="gbuf", bufs=GBUFS,
                                   padded_shape=[128, NCHMAX * SUPER * (256 // EW), EW])

                    c = 0
                    while c < nch_g and not ABL_NO_PAY:
                        b = int(chunk_bucket[gc0 + c])
                        ce = c
                        while (ce < nch_g and int(chunk_bucket[gc0 + ce]) == b
                               and ce - c < CALL_MAX):
                            ce += 1
                        k = ce - c
                        cv = int(call_valid[call_counter[0]]) if TRIM else k * 128
                        call_counter[0] += 1
                        nc.gpsimd.dma_gather(
                            gbuf[:, c:ce, :],
                            tsrc_full[b * SLAB:(b + 1) * SLAB, :],
                            pidxt[:, (gc0 + c) * 8:(gc0 + ce) * 8],
                            k * 128, cv, EW,
                            queue_num=qn % 4, single_packet=True)
                        qn += 1
                        c = ce
                    cw = 0
                    for wi in range(w, w + nwg):
                        nchw = int(nch[wi])
                        nw_nodes = min(WIN, NLOC - wi * WIN)
                        sl = slice(cw, cw + nchw)
                        # transposed one-hot [node, (chunk, lane)] for the
                        # per-edge adst matmul
                        dfT = sb.tile([128, nchw * 128], dt.int8,
                                      name=f"dfT{layer}_{wi}", tag="dfT",
                                      padded_shape=[128, NCHMAX * 128])
                        nc.sync.dma_start(
                            out=dfT[:],
                            in_=doffT_in[0:1, (gc0 + cw) * 128:(gc0 + cw + nchw) * 128]
                                .broadcast_to([128, nchw * 128]))
                        ohT = sb.tile([128, nchw * 128], dt.bfloat16,
                                      name=f"ohT{layer}_{wi}", tag="ohT",
                                      padded_shape=[128, NCHMAX * 128])
                        nc.vector.tensor_tensor(
                            out=ohT[:],
                            in0=iotP[:, 0:1].broadcast_to([128, nchw * 128]),
                            in1=dfT[:], op=mybir.AluOpType.is_equal)
                        eadst = ps.tile([128, nchw * 8], dt.float32,
                                        name=f"ea{layer}_{wi}", tag="eadst",
                                        padded_shape=[128, NCHMAX * 8])
                        for ci in range(nchw):
                            nc.tensor.matmul(
                                out=eadst[:, ci * 8:ci * 8 + 8],
                                lhsT=ohT[:, ci * 128:ci * 128 + 128],
                                rhs=tdT[:, wi * 8:wi * 8 + 8],
                                start=True, stop=True)
                        ebuf = sb.tile([128, nchw, 8], dt.float32,
                                       name=f"eb{layer}_{wi}", tag="ebuf",
                                       padded_shape=[128, NCHMAX, 8])
                        nc.vector.tensor_tensor(
                            out=ebuf[:], in0=gbuf[:, sl, CH:CH + 8],
                            in1=eadst[:].rearrange("p (c h) -> p c h", h=8),
                            op=mybir.AluOpType.add)
                        nc.scalar.activation(ebuf[:], ebuf[:],
                                             mybir.ActivationFunctionType.Prelu,
                                             alpha=NEG_SLOPE)
                        nc.scalar.activation(gbuf[:, sl, CH:CH + 8], ebuf[:],
                                             mybir.ActivationFunctionType.Exp)
                        if layer == 1:
                            nc.vector.tensor_tensor(
                                out=gbuf[:, sl, 0:CH].rearrange(
                                    "p c (h e) -> p c h e", e=8),
                                in0=gbuf[:, sl, 0:CH].rearrange(
                                    "p c (h e) -> p c h e", e=8),
                                in1=gbuf[:, sl, CH:CH + 8].unsqueeze(3)
                                    .broadcast_to([128, nchw, 8, 8]),
                                op=mybir.AluOpType.mult)
                        else:
                            nc.vector.tensor_tensor(
                                out=gbuf[:, sl, 0:CH].rearrange(
                                    "p c (e h) -> p c e h", h=8),
                                in0=gbuf[:, sl, 0:CH].rearrange(
                                    "p c (e h) -> p c e h", h=8),
                                in1=gbuf[:, sl, CH:CH + 8].unsqueeze(2)
                                    .broadcast_to([128, nchw, 20, 8]),
                                op=mybir.AluOpType.mult)
                        oh = sb.tile([128, nchw, 128], dt.bfloat16,
                                     name=f"oh{layer}_{wi}", tag="oh",
                                     padded_shape=[128, NCHMAX, 128])
                        nc.vector.tensor_tensor(
                            out=oh[:],
                            in0=dofft[:, gc0 + cw:gc0 + cw + nchw]
                                .unsqueeze(2).broadcast_to([128, nchw, 128]),
                            in1=iot[:].unsqueeze(1).broadcast_to([128, nchw, 128]),
                            op=mybir.AluOpType.is_equal)
                        pagg = ps.tile([WIN, CY], dt.float32,
                                       name=f"pg{layer}_{wi}", tag="pagg",
                                       padded_shape=[WIN, 168])
                        nmm = 1 if ABL_NO_AGG else nchw
                        for ci in range(nmm):
                            nc.tensor.matmul(
                                out=pagg[:, :],
                                lhsT=oh[:, ci, :],
                                rhs=gbuf[:, cw + ci, 0:CY],
                                start=(ci == 0), stop=(ci == nmm - 1))
                        if layer == 1:
                            epilogue1(wi, nw_nodes, pagg)
                        else:
                            epilogue2(wi, nw_nodes, pagg)
                        if slice_done is not None and wi in slice_last_w:
                            slice_done(slice_last_w.index(wi))
                        cw += nchw
                    gc0 += nch_g
                    w += nwg

            def coll2_slice(s):
                if not ABL_NO_COLL:
                    nc.gpsimd.collective_compute(
                        "AllGather", mybir.AluOpType.bypass,
                        replica_groups=[list(range(R))],
                        ins=[ts2_loc[s * SLICE_H:(s + 1) * SLICE_H]],
                        outs=[ts2_full[s * SLAB:(s + 1) * SLAB]])

            edge_phase(1, coll2_slice)
            edge_phase(2)

    nc.compile()
    return nc


def _wext(w, a_src, a_dst, out_rows, pad_cols=256, permute_ch=False, heads=8):
    """[W | W.a_src | W.a_dst] padded to [out_rows, pad_cols] f32."""
    f = w.shape[0]
    c = w.shape[1] // heads
    w3 = w.reshape(f, heads, c)
    if permute_ch:
        wmain = w3.transpose(0, 2, 1).reshape(f, heads * c)  # col = ch*H + head
    else:
        wmain = w
    was = np.einsum("fhc,hc->fh", w3, a_src)
    wad = np.einsum("fhc,hc->fh", w3, a_dst)
    out = np.zeros((out_rows, pad_cols), dtype=np.float32)
    out[:f, :w.shape[1]] = wmain
    out[:f, w.shape[1]:w.shape[1] + heads] = was
    out[:f, w.shape[1] + heads:w.shape[1] + 2 * heads] = wad
    return out


_CACHE = {}


def kernel(x, edge_index, w1, att_src1, att_dst1, b1, w2, att_src2, att_dst2, b2):
    x = np.asarray(x, dtype=np.float32)
    edge_index = np.asarray(edge_index)
    src = np.concatenate([edge_index[0], np.arange(N_NODES, dtype=np.int64)]).astype(np.int64)
    dst = np.concatenate([edge_index[1], np.arange(N_NODES, dtype=np.int64)]).astype(np.int64)

    key = hash(edge_index.tobytes())
    if key not in _CACHE:
        nch, nchwb, TCH, chunk_bucket, calls, call_valid, cores = _build_layout(src, dst)
        nkern = build_kernel(nch, nchwb, TCH, chunk_bucket, call_valid)
        _CACHE[key] = (nch, TCH, cores, nkern)
    nch, TCH, cores, nkern = _CACHE[key]

    in_maps = _prep_in_maps(x, w1, att_src1, att_dst1, b1,
                            w2, att_src2, att_dst2, b2, cores)
    res = run_bass_kernel_spmd(nkern, in_maps, core_ids=list(range(R)))
    out = np.concatenate([res.results[r]["out"] for r in range(R)], axis=0)
    return out.astype(np.float32)


def _prep_in_maps(x, w1, att_src1, att_dst1, b1, w2, att_src2, att_dst2, b2, cores):
    w1p = _wext(np.asarray(w1, np.float32), np.asarray(att_src1, np.float32),
                np.asarray(att_dst1, np.float32), 128)
    w2p = _wext(np.asarray(w2, np.float32), np.asarray(att_src2, np.float32),
                np.asarray(att_dst2, np.float32), 64, permute_ch=True)
    b1rep = np.tile(np.asarray(b1, np.float32)[None, :], (128, 1))
    b2rep = np.tile(np.asarray(b2, np.float32)[None, :], (128, 1))
    iota = np.broadcast_to(np.arange(128, dtype=np.float32), (128, 128)).astype(BF16)
    ident = np.eye(128, dtype=np.float32)
    iotP = np.arange(128, dtype=np.int8).reshape(128, 1)
    in_maps = []
    for r in range(R):
        xTr = np.ascontiguousarray(x[r * NLOC:(r + 1) * NLOC].T)
        in_maps.append({
            "xT": xTr, "w1p": w1p, "w2p": w2p, "b1rep": b1rep, "b2rep": b2rep,
            "iota": np.ascontiguousarray(iota), "ident": ident,
            "doff": cores[r]["dstoff"], "doffT": cores[r]["dstoffT"],
            "iotP": iotP, "pay_idx": cores[r]["pay_idx"],
        })
    return in_maps

